# revision 13
# baseline (speedup 1.0000x reference)
"""Trainium2 Bass kernel for nn_ActorCritic_MLPLSTM (MLP front-end + GRU +
actor/critic heads), distributed over 8 NeuronCores.

Algorithm
---------
The GRU recurrence h_t = z_t*h_{t-1} + (1-z_t)*hh_t (reset_after, sigmoid
candidate) is solved by fixed-point iteration ("scan-Picard"): each sweep
recomputes the recurrent projections rec = h_prev @ Ug for ALL timesteps as one
batched matmul, forms the gates, then solves the *diagonal linear* recurrence
h_t = a_t*h_{t-1} + b_t exactly with the hardware tensor_tensor_scan
instruction. The nonlinear fixed point converges at ~9x error reduction per
sweep. Sweeps run with fp16 matmuls (4x faster than fp32 on the PE, 10x more
accurate than bf16); the gate input projections are folded into the PSUM
accumulation with an fp16 identity matmul so the sigmoids read PSUM directly.
Final rel err ~1.5e-4 (fp16-rounding floor of the MLP front-end).

Sharding: time dimension split across 8 cores (1024 steps each) with a
192-step warmup prefix per core. The GRU's z-gate product over >=128 steps
attenuates any entry-state error to ~0, so the cores need NO communication:
each core's warmup region converges to the true trajectory before its owned
steps begin. Core 0 instead forces its known initial state via a masked fixup
of the scan coefficients at the warmup boundary.

Everything on-chip lives in "feature-major" layout [feature -> partitions,
time -> free dim], so matmuls contract over partitions and the scan runs along
the free dim. Inputs x are transposed on the host as part of sharding.
"""

import os
import sys

import numpy as np

for _p in ("/opt/trn_rl_repo", "/root/.axon_site/_ro/trn_rl_repo"):
    if os.path.isdir(_p) and _p not in sys.path:
        sys.path.append(_p)

T, D_IN, D, H, A = 8192, 2048, 512, 512, 18
N_CORES = 8
TC = T // N_CORES            # 1024 owned steps per core
WARM = 64                    # warmup prefix steps
NL = WARM + TC               # 1216 local steps per core
N_SLICES = ((0, 512), (512, 512), (1024, NL - 1024))  # (start, size) free-dim slices
N_F16_SWEEPS = 5
N_FP32_SWEEPS = 0

_cache = {}
_last_exec_ns = None
_last_scope_times = None


def _build_program():
    import concourse.bacc as bacc
    import concourse.mybir as mybir
    from concourse import tile

    f32 = mybir.dt.float32
    f16 = mybir.dt.float16
    ACT = mybir.ActivationFunctionType
    ALU = mybir.AluOpType
    AXX = mybir.AxisListType.X

    nc = bacc.Bacc("TRN2", target_bir_lowering=False, debug=False,
                   num_devices=N_CORES)

    xT_d = nc.dram_tensor("xT", [D_IN, NL], f16, kind="ExternalInput")
    W1_d = nc.dram_tensor("W1d", [D_IN, D], f16, kind="ExternalInput")
    W2_d = nc.dram_tensor("W2d", [D, D], f16, kind="ExternalInput")
    Wg_d = nc.dram_tensor("Wgd", [D, 3 * H], f16, kind="ExternalInput")
    Ug_d = nc.dram_tensor("Ugd", [H, 3 * H], f32, kind="ExternalInput")
    Ug16_d = nc.dram_tensor("Ug16d", [H, 3 * H], f16, kind="ExternalInput")
    b1_d = nc.dram_tensor("b1d", [D, 1], f32, kind="ExternalInput")
    b2_d = nc.dram_tensor("b2d", [D, 1], f32, kind="ExternalInput")
    bXp_d = nc.dram_tensor("bXpd", [3 * H, 1], f32, kind="ExternalInput")
    brh_d = nc.dram_tensor("brhd", [H, 1], f32, kind="ExternalInput")
    mf_d = nc.dram_tensor("mfd", [H, 2], f32, kind="ExternalInput")
    Wpv_d = nc.dram_tensor("Wpvd", [H, A + 1], f32, kind="ExternalInput")
    bpv_d = nc.dram_tensor("bpvd", [128, A + 1], f32, kind="ExternalInput")
    id_d = nc.dram_tensor("idd", [128, 128], f16, kind="ExternalInput")

    pol_d = nc.dram_tensor("policy", [TC, A], f32, kind="ExternalOutput")
    val_d = nc.dram_tensor("value", [TC, 1], f32, kind="ExternalOutput")
    hT_d = nc.dram_tensor("hT", [H, 1], f32, kind="ExternalOutput")

    KD = D // 128        # 4  k-tiles over D/H
    KIN = D_IN // 128    # 16 k-tiles over D_IN
    M3H = 3 * H // 128   # 12 m-tiles over 3H

    with tile.TileContext(nc) as tc:
        with tc.tile_pool(name="resident", bufs=1) as rp:
            XT = [rp.tile([128, NL], f16 if m < 2 * KD else f32,
                          name=f"XT{m}", tag=f"XT{m}") for m in range(M3H)]
            id16 = rp.tile([128, 128], f16, name="id16", tag="id16")
            nc.gpsimd.dma_start(id16[:], id_d[:])
            Wpv_sb = [rp.tile([128, A + 1], f32, name=f"wpv{k}", tag=f"wpv{k}")
                      for k in range(KD)]
            bpv_sb = rp.tile([128, A + 1], f32, name="bpv", tag="bpv")
            for k in range(KD):
                nc.gpsimd.dma_start(Wpv_sb[k][:], Wpv_d[k * 128:(k + 1) * 128, :])
            nc.gpsimd.dma_start(bpv_sb[:], bpv_d[:])

            # ---------------- Phase A: MLP front-end + gate input projections
            with nc.named_scope("phaseA"), \
                 tc.tile_pool(name="wA", bufs=1) as wA, \
                 tc.tile_pool(name="sA", bufs=6) as sA, \
                 tc.tile_pool(name="zA", bufs=2) as zA, \
                 tc.tile_pool(name="psA", bufs=1, space="PSUM") as psA:
                W1_sb = [wA.tile([128, D], f16, name=f"w1_{k}", tag=f"w1_{k}")
                         for k in range(KIN)]
                W2_sb = [wA.tile([128, D], f16, name=f"w2_{k}", tag=f"w2_{k}")
                         for k in range(KD)]
                Wg_sb = [wA.tile([128, 3 * H], f16, name=f"wg_{k}", tag=f"wg_{k}")
                         for k in range(KD)]
                b1_sb = wA.tile([128, KD], f32, name="b1", tag="b1")
                b2_sb = wA.tile([128, KD], f32, name="b2", tag="b2")
                bXp_sb = wA.tile([128, M3H], f32, name="bXp", tag="bXp")
                for k in range(KIN):
                    nc.gpsimd.dma_start(W1_sb[k][:], W1_d[k * 128:(k + 1) * 128, :])
                for k in range(KD):
                    nc.gpsimd.dma_start(b1_sb[:, k:k + 1], b1_d[k * 128:(k + 1) * 128, :])
                    nc.gpsimd.dma_start(b2_sb[:, k:k + 1], b2_d[k * 128:(k + 1) * 128, :])
                for k in range(KD):
                    nc.gpsimd.dma_start(W2_sb[k][:], W2_d[k * 128:(k + 1) * 128, :])
                    nc.gpsimd.dma_start(Wg_sb[k][:], Wg_d[k * 128:(k + 1) * 128, :])
                for m in range(M3H):
                    nc.gpsimd.dma_start(bXp_sb[:, m:m + 1], bXp_d[m * 128:(m + 1) * 128, :])

                for (st, ns) in N_SLICES:
                    # z1 = relu(W1^T xT + b1)  [D-major, ns]
                    z1s = [zA.tile([128, 512], f16, name=f"z1_{m}", tag=f"z1_{m}")
                           for m in range(KD)]
                    ps1 = [psA.tile([128, ns], f32, name=f"psA{m}", tag=f"psA{m}")
                           for m in range(KD)]
                    for k in range(KIN):
                        xt = sA.tile([128, 512], f16, name="xt", tag="xt")
                        nc.sync.dma_start(xt[:, :ns],
                                          xT_d[k * 128:(k + 1) * 128, st:st + ns])
                        for m in range(KD):
                            nc.tensor.matmul(ps1[m][:],
                                             W1_sb[k][:, m * 128:(m + 1) * 128],
                                             xt[:, :ns],
                                             start=(k == 0), stop=(k == KIN - 1))
                    for m in range(KD):
                        nc.scalar.activation(z1s[m][:, :ns], ps1[m][:], ACT.Relu,
                                             bias=b1_sb[:, m:m + 1])
                    # z2 = relu(W2^T z1 + b2)
                    z2s = [zA.tile([128, 512], f16, name=f"z2_{m}", tag=f"z2_{m}")
                           for m in range(KD)]
                    ps2 = [psA.tile([128, ns], f32, name=f"psA{m}", tag=f"psA{m}")
                           for m in range(KD)]
                    for k in range(KD):
                        for m in range(KD):
                            nc.tensor.matmul(ps2[m][:],
                                             W2_sb[k][:, m * 128:(m + 1) * 128],
                                             z1s[k][:, :ns],
                                             start=(k == 0), stop=(k == KD - 1))
                    for m in range(KD):
                        nc.scalar.activation(z2s[m][:, :ns], ps2[m][:], ACT.Relu,
                                             bias=b2_sb[:, m:m + 1])
                    # XT = Wg^T z2 + (bg0 + [bg1_zr; 0])   (two psum half-waves)
                    for half in range(2):
                        ms = range(6 * half, 6 * half + 6)
                        psX = {m: psA.tile([128, ns], f32, name=f"psA{m - 6 * half}",
                                           tag=f"psA{m - 6 * half}") for m in ms}
                        for k in range(KD):
                            for m in ms:
                                nc.tensor.matmul(psX[m][:],
                                                 Wg_sb[k][:, m * 128:(m + 1) * 128],
                                                 z2s[k][:, :ns],
                                                 start=(k == 0), stop=(k == KD - 1))
                        for m in ms:
                            nc.scalar.activation(XT[m][:, st:st + ns], psX[m][:],
                                                 ACT.Identity, bias=bXp_sb[:, m:m + 1])

            # ---------------- Phase B: scan-Picard sweeps (bf16 then fp32)
            with tc.tile_pool(name="rpB", bufs=1) as rpB, \
                 tc.tile_pool(name="sB", bufs=4) as sB, \
                 tc.tile_pool(name="sab", bufs=1) as sab, \
                 tc.tile_pool(name="srt", bufs=1) as srt, \
                 tc.tile_pool(name="psB", bufs=6, space="PSUM") as psB:
                n_hb = max(1, 2 * min(N_FP32_SWEEPS, 1))
                hb = [[rpB.tile([128, NL + 1], f32, name=f"hb{b}_{k}", tag=f"hb{b}_{k}")
                       for k in range(KD)] for b in range(n_hb)]
                hb16 = [[rpB.tile([128, NL + 1], f16, name=f"hq{b}_{k}", tag=f"hq{b}_{k}")
                         for k in range(KD)] for b in range(2)]
                Ug_sb = [rpB.tile([128, 3 * H], f32, name=f"ug{k}", tag=f"ug{k}")
                         for k in range(KD)] if N_FP32_SWEEPS else None
                Ug16 = [rpB.tile([128, 3 * H], f16, name=f"uq{k}", tag=f"uq{k}")
                        for k in range(KD)]
                brh_sb = rpB.tile([128, KD], f32, name="brh", tag="brh")
                mf_sb = rpB.tile([128, 2 * KD], f32, name="mf", tag="mf")
                for k in range(KD):
                    nc.sync.dma_start(brh_sb[:, k:k + 1], brh_d[k * 128:(k + 1) * 128, :])
                    nc.sync.dma_start(mf_sb[:, 2 * k:2 * k + 2],
                                      mf_d[k * 128:(k + 1) * 128, :])
                    nc.sync.dma_start(Ug16[k][:], Ug16_d[k * 128:(k + 1) * 128, :])
                    if N_FP32_SWEEPS:
                        nc.sync.dma_start(Ug_sb[k][:], Ug_d[k * 128:(k + 1) * 128, :])
                    nc.vector.memset(hb16[0][k][:], 0.0)
                    nc.vector.memset(hb16[1][k][:, 0:1], 0.0)
                    for b in range(n_hb):
                        nc.vector.memset(hb[b][k][:, 0:1], 0.0)

                # (input_buffer, output_buffer, matmul_weights) per sweep
                schedule = []
                for s in range(N_F16_SWEEPS):
                    inb = hb16[s % 2]
                    outb = hb16[(s + 1) % 2] if s < N_F16_SWEEPS - 1 else hb[0]
                    schedule.append((inb, outb, Ug16))
                for s in range(N_FP32_SWEEPS):
                    schedule.append((hb[s % 2], hb[(s + 1) % 2], Ug_sb))
                hfin = hb[N_FP32_SWEEPS % 2]

                for s, (cur, nxt, Ugs) in enumerate(schedule):
                    with nc.named_scope(f"sweep{s}"):
                        for (st, ns) in N_SLICES:
                            ats = {}
                            gts = {}
                            rts = {}
                            hhs = {}
                            bts = {}
                            for m in range(M3H):
                                ps = psB.tile([128, ns], f32, name="psB", tag="psB")
                                fold = m < 2 * KD
                                for k in range(KD):
                                    nc.tensor.matmul(ps[:],
                                                     Ugs[k][:, m * 128:(m + 1) * 128],
                                                     cur[k][:, st:st + ns],
                                                     start=(k == 0),
                                                     stop=(k == KD - 1 and not fold))
                                if fold:
                                    nc.tensor.matmul(ps[:], id16[:],
                                                     XT[m][:, st:st + ns],
                                                     start=False, stop=True)
                                if m < KD:        # z gate -> a coefficients
                                    a_sl = sab.tile([128, 512], f32, name=f"a{m}",
                                                    tag=f"a{m}")
                                    g_sl = sab.tile([128, 512], f32, name=f"g{m}",
                                                    tag=f"g{m}")
                                    nc.scalar.activation(a_sl[:, :ns], ps[:],
                                                         ACT.Sigmoid)
                                    nc.vector.tensor_scalar(g_sl[:, :ns], a_sl[:, :ns],
                                                            -1.0, 1.0,
                                                            op0=ALU.mult, op1=ALU.add)
                                    ats[m] = a_sl
                                    gts[m] = g_sl
                                elif m < 2 * KD:  # r gate
                                    k0 = m - KD
                                    rt = srt.tile([128, 512], f32, name=f"rt{k0}",
                                                  tag=f"rt{k0}")
                                    nc.scalar.activation(rt[:, :ns], ps[:],
                                                         ACT.Sigmoid)
                                    rts[k0] = rt
                                else:             # hh = sig(Xh + rt*(rec+brh))
                                    k0 = m - 2 * KD
                                    pre = sB.tile([128, 512], f32, name="pre", tag="pre")
                                    hh = srt.tile([128, 512], f32, name=f"hh{k0}",
                                                  tag=f"hh{k0}")
                                    nc.vector.scalar_tensor_tensor(
                                        pre[:, :ns], ps[:], brh_sb[:, k0:k0 + 1],
                                        rts[k0][:, :ns], op0=ALU.add, op1=ALU.mult)
                                    nc.vector.tensor_add(pre[:, :ns], pre[:, :ns],
                                                         XT[m][:, st:st + ns])
                                    nc.scalar.activation(hh[:, :ns], pre[:, :ns],
                                                         ACT.Sigmoid)
                                    hhs[k0] = hh
                            for k in range(KD):   # b = (1-z)*hh, 1-z = sig(-pre)
                                b_sl = sab.tile([128, 512], f32, name=f"b{k}",
                                                tag=f"b{k}")
                                nc.vector.tensor_mul(b_sl[:, :ns], gts[k][:, :ns],
                                                     hhs[k][:, :ns])
                                bts[k] = b_sl
                            if st == 0:
                                # force scan state entering the first owned step:
                                # core 0: a=0, b=h0; other cores: no-op
                                c = WARM - 1
                                for k in range(KD):
                                    nc.vector.tensor_scalar(
                                        ats[k][:, c:c + 1], ats[k][:, c:c + 1],
                                        mf_sb[:, 2 * k:2 * k + 1], None, op0=ALU.mult)
                                    nc.vector.tensor_scalar(
                                        bts[k][:, c:c + 1], bts[k][:, c:c + 1],
                                        mf_sb[:, 2 * k:2 * k + 1],
                                        mf_sb[:, 2 * k + 1:2 * k + 2],
                                        op0=ALU.mult, op1=ALU.add)
                            for k in range(KD):
                                nc.vector.tensor_tensor_scan(
                                    nxt[k][:, 1 + st:1 + st + ns],
                                    ats[k][:, :ns], bts[k][:, :ns],
                                    nxt[k][:, st:st + 1], ALU.mult, ALU.add)

                # ------------- Phase C: actor/critic heads on owned steps
                with nc.named_scope("phaseC"), \
                     tc.tile_pool(name="sC", bufs=4) as sC, \
                     tc.tile_pool(name="psC", bufs=2, space="PSUM") as psC:
                    for tt in range(TC // 128):
                        col = 1 + WARM + tt * 128
                        ps = psC.tile([128, A + 1], f32, name="psC", tag="psC")
                        for k in range(KD):
                            nc.tensor.matmul(ps[:], hfin[k][:, col:col + 128],
                                             Wpv_sb[k][:], start=(k == 0),
                                             stop=(k == KD - 1))
                        L = sC.tile([128, A + 1], f32, name="L", tag="L")
                        nc.vector.tensor_add(L[:], ps[:], bpv_sb[:])
                        nmx = sC.tile([128, 1], f32, name="nmx", tag="nmx")
                        nc.vector.reduce_max(nmx[:], L[:, :A], axis=AXX, negate=True)
                        E = sC.tile([128, A], f32, name="E", tag="E")
                        nc.scalar.activation(E[:], L[:, :A], ACT.Exp, bias=nmx[:])
                        sm = sC.tile([128, 1], f32, name="sm", tag="sm")
                        nc.vector.reduce_sum(sm[:], E[:], axis=AXX)
                        rs = sC.tile([128, 1], f32, name="rs", tag="rs")
                        nc.vector.reciprocal(rs[:], sm[:])
                        P = sC.tile([128, A], f32, name="P", tag="P")
                        nc.vector.tensor_scalar(P[:], E[:], rs[:], None, op0=ALU.mult)
                        nc.sync.dma_start(pol_d[tt * 128:(tt + 1) * 128, :], P[:])
                        nc.sync.dma_start(val_d[tt * 128:(tt + 1) * 128, :],
                                          L[:, A:A + 1])
                    for k in range(KD):
                        nc.sync.dma_start(hT_d[k * 128:(k + 1) * 128, :],
                                          hfin[k][:, NL:NL + 1])

    nc.compile()
    return nc


def kernel(x, prev_hidden, W1, b1, W2, b2, Wg, Ug, bg, Wp, bp, Wv, bv):
    global _last_exec_ns, _last_scope_times
    from concourse.bass_utils import run_bass_kernel_spmd

    x = np.ascontiguousarray(np.asarray(x, dtype=np.float32))
    prev_hidden = np.asarray(prev_hidden, dtype=np.float32)
    f32c = lambda a: np.ascontiguousarray(np.asarray(a, dtype=np.float32))
    W1, b1, W2, b2, Wg, Ug, bg, Wp, bp, Wv, bv = map(
        f32c, (W1, b1, W2, b2, Wg, Ug, bg, Wp, bp, Wv, bv))

    if "nc" not in _cache:
        _cache["nc"] = _build_program()
    nc = _cache["nc"]

    xTpad = np.zeros((D_IN, WARM + T), np.float16)
    xTpad[:, WARM:] = x.T.astype(np.float16)
    bXp = (bg[0] + np.concatenate([bg[1][:2 * H], np.zeros(H, np.float32)])
           ).reshape(3 * H, 1)
    brh = bg[1][2 * H:].reshape(H, 1)
    Wpv = np.concatenate([Wp, Wv], axis=1)
    bpv = np.tile(np.concatenate([bp, bv])[None, :], (128, 1)).astype(np.float32)

    common = {
        "W1d": W1.astype(np.float16), "W2d": W2.astype(np.float16),
        "Wgd": Wg.astype(np.float16), "Ugd": Ug,
        "Ug16d": Ug.astype(np.float16),
        "b1d": b1.reshape(D, 1), "b2d": b2.reshape(D, 1),
        "bXpd": bXp, "brhd": brh, "Wpvd": Wpv, "bpvd": bpv,
        "idd": np.eye(128, dtype=np.float16),
    }
    in_maps = []
    for c in range(N_CORES):
        mf = np.empty((H, 2), np.float32)
        if c == 0:
            mf[:, 0] = 0.0
            mf[:, 1] = prev_hidden[0]
        else:
            mf[:, 0] = 1.0
            mf[:, 1] = 0.0
        in_maps.append({
            **common,
            "xT": np.ascontiguousarray(xTpad[:, c * TC:c * TC + NL]),
            "mfd": mf,
        })

    trace = os.environ.get("BASS_KERNEL_TRACE") == "1"
    try:
        res = run_bass_kernel_spmd(nc, in_maps, list(range(N_CORES)), trace=trace)
        if trace:
            _last_exec_ns = res.exec_time_ns
            _last_scope_times = res.per_core_scope_times
        results = res.results
    except Exception:
        # Transient NRT faults (e.g. EXEC_UNIT_UNRECOVERABLE on first exec of a
        # fresh NEFF) poison the in-process PJRT client; retry in clean
        # subprocesses, which also warm the NEFF cache for a final attempt.
        results = _run_in_subprocess(in_maps)

    policy = np.concatenate([results[c]["policy"] for c in range(N_CORES)], axis=0)
    value = np.concatenate([results[c]["value"] for c in range(N_CORES)], axis=0)
    hT = results[N_CORES - 1]["hT"].reshape(1, H)
    return policy, value, hT


def _run_in_subprocess(in_maps):
    import pickle
    import subprocess
    import tempfile
    import time as _time

    last = None
    for attempt in range(3):
        with tempfile.TemporaryDirectory() as td:
            inp = os.path.join(td, "in.pkl")
            outp = os.path.join(td, "out.pkl")
            with open(inp, "wb") as f:
                pickle.dump(in_maps, f)
            code = (
                "import pickle, importlib.util, sys\n"
                f"spec = importlib.util.spec_from_file_location('knl', {__file__!r})\n"
                "m = importlib.util.module_from_spec(spec); spec.loader.exec_module(m)\n"
                f"in_maps = pickle.load(open({'PLACEIN'!r}, 'rb'))\n"
                "from concourse.bass_utils import run_bass_kernel_spmd\n"
                "nc = m._build_program()\n"
                "res = run_bass_kernel_spmd(nc, in_maps, list(range(m.N_CORES)))\n"
                f"pickle.dump(res.results, open({'PLACEOUT'!r}, 'wb'))\n"
            ).replace("PLACEIN", inp).replace("PLACEOUT", outp)
            r = subprocess.run([sys.executable, "-c", code], capture_output=True)
            if r.returncode == 0 and os.path.exists(outp):
                with open(outp, "rb") as f:
                    return pickle.load(f)
            last = r.stderr.decode(errors="replace")[-2000:]
            _time.sleep(5)
    raise RuntimeError(f"device run failed after retries: {last}")


# revision 14
# speedup vs baseline: 1.0883x; 1.0883x over previous
"""Trainium2 Bass kernel for nn_ActorCritic_MLPLSTM (MLP front-end + GRU +
actor/critic heads), distributed over 8 NeuronCores.

Algorithm
---------
The GRU recurrence h_t = z_t*h_{t-1} + (1-z_t)*hh_t (reset_after, sigmoid
candidate) is solved by fixed-point iteration ("scan-Picard"): each sweep
recomputes the recurrent projections rec = h_prev @ Ug for ALL timesteps as one
batched matmul, forms the gates, then solves the *diagonal linear* recurrence
h_t = a_t*h_{t-1} + b_t exactly with the hardware tensor_tensor_scan
instruction. The nonlinear fixed point converges at ~9x error reduction per
sweep. Sweeps run with fp16 matmuls (4x faster than fp32 on the PE, 10x more
accurate than bf16); the gate input projections are folded into the PSUM
accumulation with an fp16 identity matmul so the sigmoids read PSUM directly.
Final rel err ~1.5e-4 (fp16-rounding floor of the MLP front-end).

Sharding: time dimension split across 8 cores (1024 steps each) with a
192-step warmup prefix per core. The GRU's z-gate product over >=128 steps
attenuates any entry-state error to ~0, so the cores need NO communication:
each core's warmup region converges to the true trajectory before its owned
steps begin. Core 0 instead forces its known initial state via a masked fixup
of the scan coefficients at the warmup boundary.

Everything on-chip lives in "feature-major" layout [feature -> partitions,
time -> free dim], so matmuls contract over partitions and the scan runs along
the free dim. Inputs x are transposed on the host as part of sharding.
"""

import os
import sys

import numpy as np

for _p in ("/opt/trn_rl_repo", "/root/.axon_site/_ro/trn_rl_repo"):
    if os.path.isdir(_p) and _p not in sys.path:
        sys.path.append(_p)

T, D_IN, D, H, A = 8192, 2048, 512, 512, 18
N_CORES = 8
TC = T // N_CORES            # 1024 owned steps per core
WARM = 64                    # warmup prefix steps
NL = WARM + TC               # 1216 local steps per core
N_SLICES = ((0, 512), (512, 512), (1024, NL - 1024))  # (start, size) free-dim slices
N_F16_SWEEPS = 5
N_FP32_SWEEPS = 0

_cache = {}
_last_exec_ns = None
_last_scope_times = None


def _build_program():
    import concourse.bacc as bacc
    import concourse.mybir as mybir
    from concourse import tile

    f32 = mybir.dt.float32
    f16 = mybir.dt.float16
    ACT = mybir.ActivationFunctionType
    ALU = mybir.AluOpType
    AXX = mybir.AxisListType.X

    nc = bacc.Bacc("TRN2", target_bir_lowering=False, debug=False,
                   num_devices=N_CORES)

    xT_d = nc.dram_tensor("xT", [D_IN, NL], f16, kind="ExternalInput")
    W1_d = nc.dram_tensor("W1d", [D_IN, D], f16, kind="ExternalInput")
    W2_d = nc.dram_tensor("W2d", [D, D], f16, kind="ExternalInput")
    Wg_d = nc.dram_tensor("Wgd", [D, 3 * H], f16, kind="ExternalInput")
    Ug_d = nc.dram_tensor("Ugd", [H, 3 * H], f32, kind="ExternalInput")
    Ug16_d = nc.dram_tensor("Ug16d", [H, 3 * H], f16, kind="ExternalInput")
    b1_d = nc.dram_tensor("b1d", [D, 1], f32, kind="ExternalInput")
    b2_d = nc.dram_tensor("b2d", [D, 1], f32, kind="ExternalInput")
    bXp_d = nc.dram_tensor("bXpd", [3 * H, 1], f32, kind="ExternalInput")
    brh_d = nc.dram_tensor("brhd", [H, 1], f32, kind="ExternalInput")
    mf_d = nc.dram_tensor("mfd", [H, 2], f32, kind="ExternalInput")
    Wpv_d = nc.dram_tensor("Wpvd", [H, A + 1], f32, kind="ExternalInput")
    bpv_d = nc.dram_tensor("bpvd", [128, A + 1], f32, kind="ExternalInput")
    id_d = nc.dram_tensor("idd", [128, 128], f16, kind="ExternalInput")

    pol_d = nc.dram_tensor("policy", [TC, A], f32, kind="ExternalOutput")
    val_d = nc.dram_tensor("value", [TC, 1], f32, kind="ExternalOutput")
    hT_d = nc.dram_tensor("hT", [H, 1], f32, kind="ExternalOutput")

    KD = D // 128        # 4  k-tiles over D/H
    KIN = D_IN // 128    # 16 k-tiles over D_IN
    M3H = 3 * H // 128   # 12 m-tiles over 3H

    with tile.TileContext(nc) as tc:
        with tc.tile_pool(name="resident", bufs=1) as rp:
            XT = [rp.tile([128, NL], f16 if m < 2 * KD else f32,
                          name=f"XT{m}", tag=f"XT{m}") for m in range(M3H)]
            id16 = rp.tile([128, 128], f16, name="id16", tag="id16")
            nc.gpsimd.dma_start(id16[:], id_d[:])
            hb16 = [[rp.tile([128, NL + 1], f16, name=f"hq{b}_{k}", tag=f"hq{b}_{k}")
                     for k in range(KD)] for b in range(2)]
            brh_sb = rp.tile([128, KD], f32, name="brh", tag="brh")
            mf_sb = rp.tile([128, 2 * KD], f32, name="mf", tag="mf")
            for k in range(KD):
                nc.gpsimd.dma_start(brh_sb[:, k:k + 1], brh_d[k * 128:(k + 1) * 128, :])
                nc.gpsimd.dma_start(mf_sb[:, 2 * k:2 * k + 2],
                                    mf_d[k * 128:(k + 1) * 128, :])
                nc.vector.memset(hb16[0][k][:, 0:1], 0.0)
                nc.vector.memset(hb16[1][k][:, 0:1], 0.0)
            Wpv_sb = [rp.tile([128, A + 1], f32, name=f"wpv{k}", tag=f"wpv{k}")
                      for k in range(KD)]
            bpv_sb = rp.tile([128, A + 1], f32, name="bpv", tag="bpv")
            for k in range(KD):
                nc.gpsimd.dma_start(Wpv_sb[k][:], Wpv_d[k * 128:(k + 1) * 128, :])
            nc.gpsimd.dma_start(bpv_sb[:], bpv_d[:])

            # ---------------- Phase A: MLP front-end + gate input projections
            with nc.named_scope("phaseA"), \
                 tc.tile_pool(name="wA", bufs=1) as wA, \
                 tc.tile_pool(name="sA", bufs=6) as sA, \
                 tc.tile_pool(name="zA", bufs=2) as zA, \
                 tc.tile_pool(name="psA", bufs=1, space="PSUM") as psA:
                W1_sb = [wA.tile([128, D], f16, name=f"w1_{k}", tag=f"w1_{k}")
                         for k in range(KIN)]
                W2_sb = [wA.tile([128, D], f16, name=f"w2_{k}", tag=f"w2_{k}")
                         for k in range(KD)]
                Wg_sb = [wA.tile([128, 3 * H], f16, name=f"wg_{k}", tag=f"wg_{k}")
                         for k in range(KD)]
                b1_sb = wA.tile([128, KD], f32, name="b1", tag="b1")
                b2_sb = wA.tile([128, KD], f32, name="b2", tag="b2")
                bXp_sb = wA.tile([128, M3H], f32, name="bXp", tag="bXp")
                for k in range(KIN):
                    nc.gpsimd.dma_start(W1_sb[k][:], W1_d[k * 128:(k + 1) * 128, :])
                for k in range(KD):
                    nc.gpsimd.dma_start(b1_sb[:, k:k + 1], b1_d[k * 128:(k + 1) * 128, :])
                    nc.gpsimd.dma_start(b2_sb[:, k:k + 1], b2_d[k * 128:(k + 1) * 128, :])
                for k in range(KD):
                    nc.gpsimd.dma_start(W2_sb[k][:], W2_d[k * 128:(k + 1) * 128, :])
                    nc.gpsimd.dma_start(Wg_sb[k][:], Wg_d[k * 128:(k + 1) * 128, :])
                for m in range(M3H):
                    nc.gpsimd.dma_start(bXp_sb[:, m:m + 1], bXp_d[m * 128:(m + 1) * 128, :])

                for (st, ns) in N_SLICES:
                    # z1 = relu(W1^T xT + b1)  [D-major, ns]
                    z1s = [zA.tile([128, 512], f16, name=f"z1_{m}", tag=f"z1_{m}")
                           for m in range(KD)]
                    ps1 = [psA.tile([128, ns], f32, name=f"psA{m}", tag=f"psA{m}")
                           for m in range(KD)]
                    for k in range(KIN):
                        xt = sA.tile([128, 512], f16, name="xt", tag="xt")
                        nc.sync.dma_start(xt[:, :ns],
                                          xT_d[k * 128:(k + 1) * 128, st:st + ns])
                        for m in range(KD):
                            nc.tensor.matmul(ps1[m][:],
                                             W1_sb[k][:, m * 128:(m + 1) * 128],
                                             xt[:, :ns],
                                             start=(k == 0), stop=(k == KIN - 1))
                    for m in range(KD):
                        nc.scalar.activation(z1s[m][:, :ns], ps1[m][:], ACT.Relu,
                                             bias=b1_sb[:, m:m + 1])
                    # z2 = relu(W2^T z1 + b2)
                    z2s = [zA.tile([128, 512], f16, name=f"z2_{m}", tag=f"z2_{m}")
                           for m in range(KD)]
                    ps2 = [psA.tile([128, ns], f32, name=f"psA{m}", tag=f"psA{m}")
                           for m in range(KD)]
                    for k in range(KD):
                        for m in range(KD):
                            nc.tensor.matmul(ps2[m][:],
                                             W2_sb[k][:, m * 128:(m + 1) * 128],
                                             z1s[k][:, :ns],
                                             start=(k == 0), stop=(k == KD - 1))
                    for m in range(KD):
                        nc.scalar.activation(z2s[m][:, :ns], ps2[m][:], ACT.Relu,
                                             bias=b2_sb[:, m:m + 1])
                    # XT = Wg^T z2 + (bg0 + [bg1_zr; 0])   (two psum half-waves)
                    for half in range(2):
                        ms = range(6 * half, 6 * half + 6)
                        psX = {m: psA.tile([128, ns], f32, name=f"psA{m - 6 * half}",
                                           tag=f"psA{m - 6 * half}") for m in ms}
                        for k in range(KD):
                            for m in ms:
                                nc.tensor.matmul(psX[m][:],
                                                 Wg_sb[k][:, m * 128:(m + 1) * 128],
                                                 z2s[k][:, :ns],
                                                 start=(k == 0), stop=(k == KD - 1))
                        for m in ms:
                            nc.scalar.activation(XT[m][:, st:st + ns], psX[m][:],
                                                 ACT.Identity, bias=bXp_sb[:, m:m + 1])

                    # -------- fused sweep 0: trajectory == 0, so rec == 0 and
                    # the gates read X directly; scans write hb16[1]
                    s0a, s0g, s0r, s0h, s0b = {}, {}, {}, {}, {}
                    for k in range(KD):
                        a_sl = zA.tile([128, 512], f32, name=f"s0a{k}", tag=f"s0a{k}")
                        g_sl = zA.tile([128, 512], f32, name=f"s0g{k}", tag=f"s0g{k}")
                        nc.scalar.activation(a_sl[:, :ns], XT[k][:, st:st + ns],
                                             ACT.Sigmoid)
                        nc.vector.tensor_scalar(g_sl[:, :ns], a_sl[:, :ns], -1.0, 1.0,
                                                op0=ALU.mult, op1=ALU.add)
                        s0a[k] = a_sl
                        s0g[k] = g_sl
                        rt = zA.tile([128, 512], f32, name=f"s0r{k}", tag=f"s0r{k}")
                        nc.scalar.activation(rt[:, :ns], XT[KD + k][:, st:st + ns],
                                             ACT.Sigmoid)
                        s0r[k] = rt
                    for k in range(KD):
                        pre = zA.tile([128, 512], f32, name=f"s0p{k}", tag=f"s0p{k}")
                        hh = zA.tile([128, 512], f32, name=f"s0h{k}", tag=f"s0h{k}")
                        nc.vector.tensor_scalar(pre[:, :ns], s0r[k][:, :ns],
                                                brh_sb[:, k:k + 1], None, op0=ALU.mult)
                        nc.vector.tensor_add(pre[:, :ns], pre[:, :ns],
                                             XT[2 * KD + k][:, st:st + ns])
                        nc.scalar.activation(hh[:, :ns], pre[:, :ns], ACT.Sigmoid)
                        b_sl = zA.tile([128, 512], f32, name=f"s0b{k}", tag=f"s0b{k}")
                        nc.vector.tensor_mul(b_sl[:, :ns], s0g[k][:, :ns], hh[:, :ns])
                        s0b[k] = b_sl
                    if st == 0:
                        c = WARM - 1
                        for k in range(KD):
                            nc.vector.tensor_scalar(
                                s0a[k][:, c:c + 1], s0a[k][:, c:c + 1],
                                mf_sb[:, 2 * k:2 * k + 1], None, op0=ALU.mult)
                            nc.vector.tensor_scalar(
                                s0b[k][:, c:c + 1], s0b[k][:, c:c + 1],
                                mf_sb[:, 2 * k:2 * k + 1],
                                mf_sb[:, 2 * k + 1:2 * k + 2],
                                op0=ALU.mult, op1=ALU.add)
                    for k in range(KD):
                        nc.vector.tensor_tensor_scan(
                            hb16[1][k][:, 1 + st:1 + st + ns],
                            s0a[k][:, :ns], s0b[k][:, :ns],
                            hb16[1][k][:, st:st + 1], ALU.mult, ALU.add)

            # ---------------- Phase B: scan-Picard sweeps (bf16 then fp32)
            with tc.tile_pool(name="rpB", bufs=1) as rpB, \
                 tc.tile_pool(name="sB", bufs=4) as sB, \
                 tc.tile_pool(name="sab", bufs=1) as sab, \
                 tc.tile_pool(name="srt", bufs=1) as srt, \
                 tc.tile_pool(name="psB", bufs=6, space="PSUM") as psB:
                n_hb = max(1, 2 * min(N_FP32_SWEEPS, 1))
                hb = [[rpB.tile([128, NL + 1], f32, name=f"hb{b}_{k}", tag=f"hb{b}_{k}")
                       for k in range(KD)] for b in range(n_hb)]
                Ug_sb = [rpB.tile([128, 3 * H], f32, name=f"ug{k}", tag=f"ug{k}")
                         for k in range(KD)] if N_FP32_SWEEPS else None
                Ug16 = [rpB.tile([128, 3 * H], f16, name=f"uq{k}", tag=f"uq{k}")
                        for k in range(KD)]
                for k in range(KD):
                    nc.sync.dma_start(Ug16[k][:], Ug16_d[k * 128:(k + 1) * 128, :])
                    if N_FP32_SWEEPS:
                        nc.sync.dma_start(Ug_sb[k][:], Ug_d[k * 128:(k + 1) * 128, :])
                    for b in range(n_hb):
                        nc.vector.memset(hb[b][k][:, 0:1], 0.0)

                # (input_buffer, output_buffer, matmul_weights) per sweep
                schedule = []
                for s in range(1, N_F16_SWEEPS):
                    inb = hb16[s % 2]
                    outb = hb16[(s + 1) % 2] if s < N_F16_SWEEPS - 1 else hb[0]
                    schedule.append((s, inb, outb, Ug16))
                for s in range(N_FP32_SWEEPS):
                    schedule.append((N_F16_SWEEPS + s, hb[s % 2], hb[(s + 1) % 2], Ug_sb))
                hfin = hb[N_FP32_SWEEPS % 2]

                for s, cur, nxt, Ugs in schedule:
                    with nc.named_scope(f"sweep{s}"):
                        for (st, ns) in N_SLICES:
                            ats = {}
                            gts = {}
                            rts = {}
                            hhs = {}
                            bts = {}
                            for m in range(M3H):
                                ps = psB.tile([128, ns], f32, name="psB", tag="psB")
                                fold = m < 2 * KD
                                for k in range(KD):
                                    nc.tensor.matmul(ps[:],
                                                     Ugs[k][:, m * 128:(m + 1) * 128],
                                                     cur[k][:, st:st + ns],
                                                     start=(k == 0),
                                                     stop=(k == KD - 1 and not fold))
                                if fold:
                                    nc.tensor.matmul(ps[:], id16[:],
                                                     XT[m][:, st:st + ns],
                                                     start=False, stop=True)
                                if m < KD:        # z gate -> a coefficients
                                    a_sl = sab.tile([128, 512], f32, name=f"a{m}",
                                                    tag=f"a{m}")
                                    g_sl = sab.tile([128, 512], f32, name=f"g{m}",
                                                    tag=f"g{m}")
                                    nc.scalar.activation(a_sl[:, :ns], ps[:],
                                                         ACT.Sigmoid)
                                    nc.vector.tensor_scalar(g_sl[:, :ns], a_sl[:, :ns],
                                                            -1.0, 1.0,
                                                            op0=ALU.mult, op1=ALU.add)
                                    ats[m] = a_sl
                                    gts[m] = g_sl
                                elif m < 2 * KD:  # r gate
                                    k0 = m - KD
                                    rt = srt.tile([128, 512], f32, name=f"rt{k0}",
                                                  tag=f"rt{k0}")
                                    nc.scalar.activation(rt[:, :ns], ps[:],
                                                         ACT.Sigmoid)
                                    rts[k0] = rt
                                else:             # hh = sig(Xh + rt*(rec+brh))
                                    k0 = m - 2 * KD
                                    pre = sB.tile([128, 512], f32, name="pre", tag="pre")
                                    hh = srt.tile([128, 512], f32, name=f"hh{k0}",
                                                  tag=f"hh{k0}")
                                    nc.vector.scalar_tensor_tensor(
                                        pre[:, :ns], ps[:], brh_sb[:, k0:k0 + 1],
                                        rts[k0][:, :ns], op0=ALU.add, op1=ALU.mult)
                                    nc.vector.tensor_add(pre[:, :ns], pre[:, :ns],
                                                         XT[m][:, st:st + ns])
                                    nc.scalar.activation(hh[:, :ns], pre[:, :ns],
                                                         ACT.Sigmoid)
                                    hhs[k0] = hh
                            for k in range(KD):   # b = (1-z)*hh, 1-z = sig(-pre)
                                b_sl = sab.tile([128, 512], f32, name=f"b{k}",
                                                tag=f"b{k}")
                                nc.vector.tensor_mul(b_sl[:, :ns], gts[k][:, :ns],
                                                     hhs[k][:, :ns])
                                bts[k] = b_sl
                            if st == 0:
                                # force scan state entering the first owned step:
                                # core 0: a=0, b=h0; other cores: no-op
                                c = WARM - 1
                                for k in range(KD):
                                    nc.vector.tensor_scalar(
                                        ats[k][:, c:c + 1], ats[k][:, c:c + 1],
                                        mf_sb[:, 2 * k:2 * k + 1], None, op0=ALU.mult)
                                    nc.vector.tensor_scalar(
                                        bts[k][:, c:c + 1], bts[k][:, c:c + 1],
                                        mf_sb[:, 2 * k:2 * k + 1],
                                        mf_sb[:, 2 * k + 1:2 * k + 2],
                                        op0=ALU.mult, op1=ALU.add)
                            for k in range(KD):
                                nc.vector.tensor_tensor_scan(
                                    nxt[k][:, 1 + st:1 + st + ns],
                                    ats[k][:, :ns], bts[k][:, :ns],
                                    nxt[k][:, st:st + 1], ALU.mult, ALU.add)

                # ------------- Phase C: actor/critic heads on owned steps
                with nc.named_scope("phaseC"), \
                     tc.tile_pool(name="sC", bufs=4) as sC, \
                     tc.tile_pool(name="psC", bufs=2, space="PSUM") as psC:
                    for tt in range(TC // 128):
                        col = 1 + WARM + tt * 128
                        ps = psC.tile([128, A + 1], f32, name="psC", tag="psC")
                        for k in range(KD):
                            nc.tensor.matmul(ps[:], hfin[k][:, col:col + 128],
                                             Wpv_sb[k][:], start=(k == 0),
                                             stop=(k == KD - 1))
                        L = sC.tile([128, A + 1], f32, name="L", tag="L")
                        nc.vector.tensor_add(L[:], ps[:], bpv_sb[:])
                        nmx = sC.tile([128, 1], f32, name="nmx", tag="nmx")
                        nc.vector.reduce_max(nmx[:], L[:, :A], axis=AXX, negate=True)
                        E = sC.tile([128, A], f32, name="E", tag="E")
                        nc.scalar.activation(E[:], L[:, :A], ACT.Exp, bias=nmx[:])
                        sm = sC.tile([128, 1], f32, name="sm", tag="sm")
                        nc.vector.reduce_sum(sm[:], E[:], axis=AXX)
                        rs = sC.tile([128, 1], f32, name="rs", tag="rs")
                        nc.vector.reciprocal(rs[:], sm[:])
                        P = sC.tile([128, A], f32, name="P", tag="P")
                        nc.vector.tensor_scalar(P[:], E[:], rs[:], None, op0=ALU.mult)
                        nc.sync.dma_start(pol_d[tt * 128:(tt + 1) * 128, :], P[:])
                        nc.sync.dma_start(val_d[tt * 128:(tt + 1) * 128, :],
                                          L[:, A:A + 1])
                    for k in range(KD):
                        nc.sync.dma_start(hT_d[k * 128:(k + 1) * 128, :],
                                          hfin[k][:, NL:NL + 1])

    nc.compile()
    return nc


def kernel(x, prev_hidden, W1, b1, W2, b2, Wg, Ug, bg, Wp, bp, Wv, bv):
    global _last_exec_ns, _last_scope_times
    from concourse.bass_utils import run_bass_kernel_spmd

    x = np.ascontiguousarray(np.asarray(x, dtype=np.float32))
    prev_hidden = np.asarray(prev_hidden, dtype=np.float32)
    f32c = lambda a: np.ascontiguousarray(np.asarray(a, dtype=np.float32))
    W1, b1, W2, b2, Wg, Ug, bg, Wp, bp, Wv, bv = map(
        f32c, (W1, b1, W2, b2, Wg, Ug, bg, Wp, bp, Wv, bv))

    if "nc" not in _cache:
        _cache["nc"] = _build_program()
    nc = _cache["nc"]

    xTpad = np.zeros((D_IN, WARM + T), np.float16)
    xTpad[:, WARM:] = x.T.astype(np.float16)
    bXp = (bg[0] + np.concatenate([bg[1][:2 * H], np.zeros(H, np.float32)])
           ).reshape(3 * H, 1)
    brh = bg[1][2 * H:].reshape(H, 1)
    Wpv = np.concatenate([Wp, Wv], axis=1)
    bpv = np.tile(np.concatenate([bp, bv])[None, :], (128, 1)).astype(np.float32)

    common = {
        "W1d": W1.astype(np.float16), "W2d": W2.astype(np.float16),
        "Wgd": Wg.astype(np.float16), "Ugd": Ug,
        "Ug16d": Ug.astype(np.float16),
        "b1d": b1.reshape(D, 1), "b2d": b2.reshape(D, 1),
        "bXpd": bXp, "brhd": brh, "Wpvd": Wpv, "bpvd": bpv,
        "idd": np.eye(128, dtype=np.float16),
    }
    in_maps = []
    for c in range(N_CORES):
        mf = np.empty((H, 2), np.float32)
        if c == 0:
            mf[:, 0] = 0.0
            mf[:, 1] = prev_hidden[0]
        else:
            mf[:, 0] = 1.0
            mf[:, 1] = 0.0
        in_maps.append({
            **common,
            "xT": np.ascontiguousarray(xTpad[:, c * TC:c * TC + NL]),
            "mfd": mf,
        })

    trace = os.environ.get("BASS_KERNEL_TRACE") == "1"
    try:
        res = run_bass_kernel_spmd(nc, in_maps, list(range(N_CORES)), trace=trace)
        if trace:
            _last_exec_ns = res.exec_time_ns
            _last_scope_times = res.per_core_scope_times
        results = res.results
    except Exception:
        # Transient NRT faults (e.g. EXEC_UNIT_UNRECOVERABLE on first exec of a
        # fresh NEFF) poison the in-process PJRT client; retry in clean
        # subprocesses, which also warm the NEFF cache for a final attempt.
        results = _run_in_subprocess(in_maps)

    policy = np.concatenate([results[c]["policy"] for c in range(N_CORES)], axis=0)
    value = np.concatenate([results[c]["value"] for c in range(N_CORES)], axis=0)
    hT = results[N_CORES - 1]["hT"].reshape(1, H)
    return policy, value, hT


def _run_in_subprocess(in_maps):
    import pickle
    import subprocess
    import tempfile
    import time as _time

    last = None
    for attempt in range(3):
        with tempfile.TemporaryDirectory() as td:
            inp = os.path.join(td, "in.pkl")
            outp = os.path.join(td, "out.pkl")
            with open(inp, "wb") as f:
                pickle.dump(in_maps, f)
            code = (
                "import pickle, importlib.util, sys\n"
                f"spec = importlib.util.spec_from_file_location('knl', {__file__!r})\n"
                "m = importlib.util.module_from_spec(spec); spec.loader.exec_module(m)\n"
                f"in_maps = pickle.load(open({'PLACEIN'!r}, 'rb'))\n"
                "from concourse.bass_utils import run_bass_kernel_spmd\n"
                "nc = m._build_program()\n"
                "res = run_bass_kernel_spmd(nc, in_maps, list(range(m.N_CORES)))\n"
                f"pickle.dump(res.results, open({'PLACEOUT'!r}, 'wb'))\n"
            ).replace("PLACEIN", inp).replace("PLACEOUT", outp)
            r = subprocess.run([sys.executable, "-c", code], capture_output=True)
            if r.returncode == 0 and os.path.exists(outp):
                with open(outp, "rb") as f:
                    return pickle.load(f)
            last = r.stderr.decode(errors="replace")[-2000:]
            _time.sleep(5)
    raise RuntimeError(f"device run failed after retries: {last}")


# revision 15
# speedup vs baseline: 1.2486x; 1.1473x over previous
"""Trainium2 Bass kernel for nn_ActorCritic_MLPLSTM (MLP front-end + GRU +
actor/critic heads), distributed over 8 NeuronCores.

Algorithm
---------
The GRU recurrence h_t = z_t*h_{t-1} + (1-z_t)*hh_t (reset_after, sigmoid
candidate) is solved by fixed-point iteration ("scan-Picard"): each sweep
recomputes the recurrent projections rec = h_prev @ Ug for ALL timesteps as one
batched matmul, forms the gates, then solves the *diagonal linear* recurrence
h_t = a_t*h_{t-1} + b_t exactly with the hardware tensor_tensor_scan
instruction. The nonlinear fixed point converges at ~9x error reduction per
sweep. Sweeps run with fp16 matmuls (4x faster than fp32 on the PE, 10x more
accurate than bf16); the gate input projections are folded into the PSUM
accumulation with an fp16 identity matmul so the sigmoids read PSUM directly.
Final rel err ~1.5e-4 (fp16-rounding floor of the MLP front-end).

Sharding: time dimension split across 8 cores (1024 steps each) with a
192-step warmup prefix per core. The GRU's z-gate product over >=128 steps
attenuates any entry-state error to ~0, so the cores need NO communication:
each core's warmup region converges to the true trajectory before its owned
steps begin. Core 0 instead forces its known initial state via a masked fixup
of the scan coefficients at the warmup boundary.

Everything on-chip lives in "feature-major" layout [feature -> partitions,
time -> free dim], so matmuls contract over partitions and the scan runs along
the free dim. Inputs x are transposed on the host as part of sharding.
"""

import os
import sys

import numpy as np

for _p in ("/opt/trn_rl_repo", "/root/.axon_site/_ro/trn_rl_repo"):
    if os.path.isdir(_p) and _p not in sys.path:
        sys.path.append(_p)

T, D_IN, D, H, A = 8192, 2048, 512, 512, 18
N_CORES = 8
TC = T // N_CORES            # 1024 owned steps per core
WARM = 64                    # warmup prefix steps
NL = WARM + TC               # 1216 local steps per core
N_SLICES = ((0, 512), (512, 512), (1024, NL - 1024))  # (start, size) free-dim slices
N_F16_SWEEPS = 5
N_FP32_SWEEPS = 0

_cache = {}
_last_exec_ns = None
_last_scope_times = None


def _build_program():
    import concourse.bacc as bacc
    import concourse.mybir as mybir
    from concourse import tile

    f32 = mybir.dt.float32
    f16 = mybir.dt.float16
    ACT = mybir.ActivationFunctionType
    ALU = mybir.AluOpType
    AXX = mybir.AxisListType.X

    nc = bacc.Bacc("TRN2", target_bir_lowering=False, debug=False,
                   num_devices=N_CORES)

    xT_d = nc.dram_tensor("xT", [D_IN, NL], f16, kind="ExternalInput")
    W1_d = nc.dram_tensor("W1d", [D_IN, D], f16, kind="ExternalInput")
    W2_d = nc.dram_tensor("W2d", [D, D], f16, kind="ExternalInput")
    Wg_d = nc.dram_tensor("Wgd", [D, 3 * H], f16, kind="ExternalInput")
    Ug_d = nc.dram_tensor("Ugd", [H, 3 * H], f32, kind="ExternalInput")
    Ug16_d = nc.dram_tensor("Ug16d", [H, 3 * H], f16, kind="ExternalInput")
    b1_d = nc.dram_tensor("b1d", [D, 1], f32, kind="ExternalInput")
    b2_d = nc.dram_tensor("b2d", [D, 1], f32, kind="ExternalInput")
    bXp_d = nc.dram_tensor("bXpd", [3 * H, 1], f32, kind="ExternalInput")
    brh_d = nc.dram_tensor("brhd", [H, 1], f32, kind="ExternalInput")
    mf_d = nc.dram_tensor("mfd", [H, 2], f32, kind="ExternalInput")
    Wpv_d = nc.dram_tensor("Wpvd", [H, A + 1], f32, kind="ExternalInput")
    bpv_d = nc.dram_tensor("bpvd", [128, A + 1], f32, kind="ExternalInput")
    id_d = nc.dram_tensor("idd", [128, 128], f16, kind="ExternalInput")

    pol_d = nc.dram_tensor("policy", [TC, A], f32, kind="ExternalOutput")
    val_d = nc.dram_tensor("value", [TC, 1], f32, kind="ExternalOutput")
    hT_d = nc.dram_tensor("hT", [H, 1], f32, kind="ExternalOutput")

    KD = D // 128        # 4  k-tiles over D/H
    KIN = D_IN // 128    # 16 k-tiles over D_IN
    M3H = 3 * H // 128   # 12 m-tiles over 3H

    with tile.TileContext(nc) as tc:
        with tc.tile_pool(name="resident", bufs=1) as rp:
            XT = [rp.tile([128, NL], f16 if m < 2 * KD else f32,
                          name=f"XT{m}", tag=f"XT{m}") for m in range(M3H)]
            id16 = rp.tile([128, 128], f16, name="id16", tag="id16")
            nc.gpsimd.dma_start(id16[:], id_d[:])
            hb16 = [[rp.tile([128, NL + 1], f16, name=f"hq{b}_{k}", tag=f"hq{b}_{k}")
                     for k in range(KD)] for b in range(2)]
            brh_sb = rp.tile([128, KD], f32, name="brh", tag="brh")
            mf_sb = rp.tile([128, 2 * KD], f32, name="mf", tag="mf")
            for k in range(KD):
                nc.gpsimd.dma_start(brh_sb[:, k:k + 1], brh_d[k * 128:(k + 1) * 128, :])
                nc.gpsimd.dma_start(mf_sb[:, 2 * k:2 * k + 2],
                                    mf_d[k * 128:(k + 1) * 128, :])
                nc.vector.memset(hb16[0][k][:, 0:1], 0.0)
                nc.vector.memset(hb16[1][k][:, 0:1], 0.0)
            Wpv_sb = [rp.tile([128, A + 1], f32, name=f"wpv{k}", tag=f"wpv{k}")
                      for k in range(KD)]
            bpv_sb = rp.tile([128, A + 1], f32, name="bpv", tag="bpv")
            for k in range(KD):
                nc.gpsimd.dma_start(Wpv_sb[k][:], Wpv_d[k * 128:(k + 1) * 128, :])
            nc.gpsimd.dma_start(bpv_sb[:], bpv_d[:])

            # ---------------- Phase A: MLP front-end + gate input projections
            with nc.named_scope("phaseA"), \
                 tc.tile_pool(name="wA", bufs=1) as wA, \
                 tc.tile_pool(name="sA", bufs=6) as sA, \
                 tc.tile_pool(name="zA", bufs=2) as zA, \
                 tc.tile_pool(name="psA", bufs=1, space="PSUM") as psA:
                W1_sb = [wA.tile([128, D], f16, name=f"w1_{k}", tag=f"w1_{k}")
                         for k in range(KIN)]
                W2_sb = [wA.tile([128, D], f16, name=f"w2_{k}", tag=f"w2_{k}")
                         for k in range(KD)]
                Wg_sb = [wA.tile([128, 3 * H], f16, name=f"wg_{k}", tag=f"wg_{k}")
                         for k in range(KD)]
                b1_sb = wA.tile([128, KD], f32, name="b1", tag="b1")
                b2_sb = wA.tile([128, KD], f32, name="b2", tag="b2")
                bXp_sb = wA.tile([128, M3H], f32, name="bXp", tag="bXp")
                for k in range(KIN):
                    nc.sync.dma_start(W1_sb[k][:], W1_d[k * 128:(k + 1) * 128, :])
                for k in range(KD):
                    nc.sync.dma_start(b1_sb[:, k:k + 1], b1_d[k * 128:(k + 1) * 128, :])
                    nc.gpsimd.dma_start(b2_sb[:, k:k + 1], b2_d[k * 128:(k + 1) * 128, :])
                for k in range(KD):
                    nc.gpsimd.dma_start(W2_sb[k][:], W2_d[k * 128:(k + 1) * 128, :])
                    nc.gpsimd.dma_start(Wg_sb[k][:], Wg_d[k * 128:(k + 1) * 128, :])
                for m in range(M3H):
                    nc.gpsimd.dma_start(bXp_sb[:, m:m + 1], bXp_d[m * 128:(m + 1) * 128, :])

                for (st, ns) in N_SLICES:
                    # z1 = relu(W1^T xT + b1)  [D-major, ns]
                    z1s = [zA.tile([128, 512], f16, name=f"z1_{m}", tag=f"z1_{m}")
                           for m in range(KD)]
                    ps1 = [psA.tile([128, ns], f32, name=f"psA{m}", tag=f"psA{m}")
                           for m in range(KD)]
                    for k in range(KIN):
                        xt = sA.tile([128, 512], f16, name="xt", tag="xt")
                        nc.sync.dma_start(xt[:, :ns],
                                          xT_d[k * 128:(k + 1) * 128, st:st + ns])
                        for m in range(KD):
                            nc.tensor.matmul(ps1[m][:],
                                             W1_sb[k][:, m * 128:(m + 1) * 128],
                                             xt[:, :ns],
                                             start=(k == 0), stop=(k == KIN - 1))
                    for m in range(KD):
                        nc.scalar.activation(z1s[m][:, :ns], ps1[m][:], ACT.Relu,
                                             bias=b1_sb[:, m:m + 1])
                    # z2 = relu(W2^T z1 + b2)
                    z2s = [zA.tile([128, 512], f16, name=f"z2_{m}", tag=f"z2_{m}")
                           for m in range(KD)]
                    ps2 = [psA.tile([128, ns], f32, name=f"psA{m}", tag=f"psA{m}")
                           for m in range(KD)]
                    for k in range(KD):
                        for m in range(KD):
                            nc.tensor.matmul(ps2[m][:],
                                             W2_sb[k][:, m * 128:(m + 1) * 128],
                                             z1s[k][:, :ns],
                                             start=(k == 0), stop=(k == KD - 1))
                    for m in range(KD):
                        nc.scalar.activation(z2s[m][:, :ns], ps2[m][:], ACT.Relu,
                                             bias=b2_sb[:, m:m + 1])
                    # XT = Wg^T z2 + (bg0 + [bg1_zr; 0])   (two psum half-waves)
                    for half in range(2):
                        ms = range(6 * half, 6 * half + 6)
                        psX = {m: psA.tile([128, ns], f32, name=f"psA{m - 6 * half}",
                                           tag=f"psA{m - 6 * half}") for m in ms}
                        for k in range(KD):
                            for m in ms:
                                nc.tensor.matmul(psX[m][:],
                                                 Wg_sb[k][:, m * 128:(m + 1) * 128],
                                                 z2s[k][:, :ns],
                                                 start=(k == 0), stop=(k == KD - 1))
                        for m in ms:
                            nc.scalar.activation(XT[m][:, st:st + ns], psX[m][:],
                                                 ACT.Identity, bias=bXp_sb[:, m:m + 1])

                    # -------- fused sweep 0: trajectory == 0, so rec == 0 and
                    # the gates read X directly; scans write hb16[1]
                    s0a, s0g, s0r, s0h, s0b = {}, {}, {}, {}, {}
                    for k in range(KD):
                        a_sl = zA.tile([128, 512], f32, name=f"s0a{k}", tag=f"s0a{k}")
                        g_sl = zA.tile([128, 512], f32, name=f"s0g{k}", tag=f"s0g{k}")
                        nc.scalar.activation(a_sl[:, :ns], XT[k][:, st:st + ns],
                                             ACT.Sigmoid)
                        nc.vector.tensor_scalar(g_sl[:, :ns], a_sl[:, :ns], -1.0, 1.0,
                                                op0=ALU.mult, op1=ALU.add)
                        s0a[k] = a_sl
                        s0g[k] = g_sl
                        rt = zA.tile([128, 512], f32, name=f"s0r{k}", tag=f"s0r{k}")
                        nc.scalar.activation(rt[:, :ns], XT[KD + k][:, st:st + ns],
                                             ACT.Sigmoid)
                        s0r[k] = rt
                    for k in range(KD):
                        pre = zA.tile([128, 512], f32, name=f"s0p{k}", tag=f"s0p{k}")
                        hh = zA.tile([128, 512], f32, name=f"s0h{k}", tag=f"s0h{k}")
                        nc.vector.tensor_scalar(pre[:, :ns], s0r[k][:, :ns],
                                                brh_sb[:, k:k + 1], None, op0=ALU.mult)
                        nc.vector.tensor_add(pre[:, :ns], pre[:, :ns],
                                             XT[2 * KD + k][:, st:st + ns])
                        nc.scalar.activation(hh[:, :ns], pre[:, :ns], ACT.Sigmoid)
                        b_sl = zA.tile([128, 512], f32, name=f"s0b{k}", tag=f"s0b{k}")
                        nc.vector.tensor_mul(b_sl[:, :ns], s0g[k][:, :ns], hh[:, :ns])
                        s0b[k] = b_sl
                    if st == 0:
                        c = WARM - 1
                        for k in range(KD):
                            nc.vector.tensor_scalar(
                                s0a[k][:, c:c + 1], s0a[k][:, c:c + 1],
                                mf_sb[:, 2 * k:2 * k + 1], None, op0=ALU.mult)
                            nc.vector.tensor_scalar(
                                s0b[k][:, c:c + 1], s0b[k][:, c:c + 1],
                                mf_sb[:, 2 * k:2 * k + 1],
                                mf_sb[:, 2 * k + 1:2 * k + 2],
                                op0=ALU.mult, op1=ALU.add)
                    for k in range(KD):
                        nc.vector.tensor_tensor_scan(
                            hb16[1][k][:, 1 + st:1 + st + ns],
                            s0a[k][:, :ns], s0b[k][:, :ns],
                            hb16[1][k][:, st:st + 1], ALU.mult, ALU.add)

            # ---------------- Phase B: scan-Picard sweeps (bf16 then fp32)
            with tc.tile_pool(name="rpB", bufs=1) as rpB, \
                 tc.tile_pool(name="sB", bufs=4) as sB, \
                 tc.tile_pool(name="sab", bufs=1) as sab, \
                 tc.tile_pool(name="srt", bufs=1) as srt, \
                 tc.tile_pool(name="psB", bufs=7, space="PSUM") as psB:
                n_hb = max(1, 2 * min(N_FP32_SWEEPS, 1))
                hb = [[rpB.tile([128, NL + 1], f32, name=f"hb{b}_{k}", tag=f"hb{b}_{k}")
                       for k in range(KD)] for b in range(n_hb)]
                Ug_sb = [rpB.tile([128, 3 * H], f32, name=f"ug{k}", tag=f"ug{k}")
                         for k in range(KD)] if N_FP32_SWEEPS else None
                Ug16 = [rpB.tile([128, 3 * H], f16, name=f"uq{k}", tag=f"uq{k}")
                        for k in range(KD)]
                for k in range(KD):
                    nc.sync.dma_start(Ug16[k][:], Ug16_d[k * 128:(k + 1) * 128, :])
                    if N_FP32_SWEEPS:
                        nc.sync.dma_start(Ug_sb[k][:], Ug_d[k * 128:(k + 1) * 128, :])
                    for b in range(n_hb):
                        nc.vector.memset(hb[b][k][:, 0:1], 0.0)

                # (input_buffer, output_buffer, matmul_weights) per sweep
                schedule = []
                for s in range(1, N_F16_SWEEPS):
                    inb = hb16[s % 2]
                    outb = hb16[(s + 1) % 2] if s < N_F16_SWEEPS - 1 else hb[0]
                    schedule.append((s, inb, outb, Ug16))
                for s in range(N_FP32_SWEEPS):
                    schedule.append((N_F16_SWEEPS + s, hb[s % 2], hb[(s + 1) % 2], Ug_sb))
                hfin = hb[N_FP32_SWEEPS % 2]

                for s, cur, nxt, Ugs in schedule:
                    with nc.named_scope(f"sweep{s}"):
                        for (st, ns) in N_SLICES:
                            ats = {}
                            gts = {}
                            rts = {}
                            hhs = {}
                            bts = {}
                            for m in range(M3H):
                                ps = psB.tile([128, ns], f32, name="psB", tag="psB")
                                fold = m < 2 * KD
                                for k in range(KD):
                                    nc.tensor.matmul(ps[:],
                                                     Ugs[k][:, m * 128:(m + 1) * 128],
                                                     cur[k][:, st:st + ns],
                                                     start=(k == 0),
                                                     stop=(k == KD - 1 and not fold))
                                if fold:
                                    nc.tensor.matmul(ps[:], id16[:],
                                                     XT[m][:, st:st + ns],
                                                     start=False, stop=True)
                                if m < KD:        # z gate -> a coefficients
                                    a_sl = sab.tile([128, 512], f32, name=f"a{m}",
                                                    tag=f"a{m}")
                                    g_sl = sab.tile([128, 512], f32, name=f"g{m}",
                                                    tag=f"g{m}")
                                    nc.scalar.activation(a_sl[:, :ns], ps[:],
                                                         ACT.Sigmoid)
                                    nc.vector.tensor_scalar(g_sl[:, :ns], a_sl[:, :ns],
                                                            -1.0, 1.0,
                                                            op0=ALU.mult, op1=ALU.add)
                                    ats[m] = a_sl
                                    gts[m] = g_sl
                                elif m < 2 * KD:  # r gate
                                    k0 = m - KD
                                    rt = srt.tile([128, 512], f32, name=f"rt{k0}",
                                                  tag=f"rt{k0}")
                                    nc.scalar.activation(rt[:, :ns], ps[:],
                                                         ACT.Sigmoid)
                                    rts[k0] = rt
                                else:             # hh = sig(Xh + rt*(rec+brh))
                                    k0 = m - 2 * KD
                                    pre = sB.tile([128, 512], f32, name="pre", tag="pre")
                                    hh = srt.tile([128, 512], f32, name=f"hh{k0}",
                                                  tag=f"hh{k0}")
                                    nc.vector.scalar_tensor_tensor(
                                        pre[:, :ns], ps[:], brh_sb[:, k0:k0 + 1],
                                        rts[k0][:, :ns], op0=ALU.add, op1=ALU.mult)
                                    nc.vector.tensor_add(pre[:, :ns], pre[:, :ns],
                                                         XT[m][:, st:st + ns])
                                    nc.scalar.activation(hh[:, :ns], pre[:, :ns],
                                                         ACT.Sigmoid)
                                    hhs[k0] = hh
                            for k in range(KD):   # b = (1-z)*hh, 1-z = sig(-pre)
                                b_sl = sab.tile([128, 512], f32, name=f"b{k}",
                                                tag=f"b{k}")
                                nc.vector.tensor_mul(b_sl[:, :ns], gts[k][:, :ns],
                                                     hhs[k][:, :ns])
                                bts[k] = b_sl
                            if st == 0:
                                # force scan state entering the first owned step:
                                # core 0: a=0, b=h0; other cores: no-op
                                c = WARM - 1
                                for k in range(KD):
                                    nc.vector.tensor_scalar(
                                        ats[k][:, c:c + 1], ats[k][:, c:c + 1],
                                        mf_sb[:, 2 * k:2 * k + 1], None, op0=ALU.mult)
                                    nc.vector.tensor_scalar(
                                        bts[k][:, c:c + 1], bts[k][:, c:c + 1],
                                        mf_sb[:, 2 * k:2 * k + 1],
                                        mf_sb[:, 2 * k + 1:2 * k + 2],
                                        op0=ALU.mult, op1=ALU.add)
                            for k in range(KD):
                                nc.vector.tensor_tensor_scan(
                                    nxt[k][:, 1 + st:1 + st + ns],
                                    ats[k][:, :ns], bts[k][:, :ns],
                                    nxt[k][:, st:st + 1], ALU.mult, ALU.add)

                # ------------- Phase C: actor/critic heads on owned steps
                with nc.named_scope("phaseC"), \
                     tc.tile_pool(name="sC", bufs=4) as sC, \
                     tc.tile_pool(name="psC", bufs=1, space="PSUM") as psC:
                    for tt in range(TC // 128):
                        col = 1 + WARM + tt * 128
                        ps = psC.tile([128, A + 1], f32, name="psC", tag="psC")
                        for k in range(KD):
                            nc.tensor.matmul(ps[:], hfin[k][:, col:col + 128],
                                             Wpv_sb[k][:], start=(k == 0),
                                             stop=(k == KD - 1))
                        L = sC.tile([128, A + 1], f32, name="L", tag="L")
                        nc.vector.tensor_add(L[:], ps[:], bpv_sb[:])
                        nmx = sC.tile([128, 1], f32, name="nmx", tag="nmx")
                        nc.vector.reduce_max(nmx[:], L[:, :A], axis=AXX, negate=True)
                        E = sC.tile([128, A], f32, name="E", tag="E")
                        nc.scalar.activation(E[:], L[:, :A], ACT.Exp, bias=nmx[:])
                        sm = sC.tile([128, 1], f32, name="sm", tag="sm")
                        nc.vector.reduce_sum(sm[:], E[:], axis=AXX)
                        rs = sC.tile([128, 1], f32, name="rs", tag="rs")
                        nc.vector.reciprocal(rs[:], sm[:])
                        P = sC.tile([128, A], f32, name="P", tag="P")
                        nc.vector.tensor_scalar(P[:], E[:], rs[:], None, op0=ALU.mult)
                        nc.sync.dma_start(pol_d[tt * 128:(tt + 1) * 128, :], P[:])
                        nc.sync.dma_start(val_d[tt * 128:(tt + 1) * 128, :],
                                          L[:, A:A + 1])
                    for k in range(KD):
                        nc.sync.dma_start(hT_d[k * 128:(k + 1) * 128, :],
                                          hfin[k][:, NL:NL + 1])

    nc.compile()
    return nc


def kernel(x, prev_hidden, W1, b1, W2, b2, Wg, Ug, bg, Wp, bp, Wv, bv):
    global _last_exec_ns, _last_scope_times
    from concourse.bass_utils import run_bass_kernel_spmd

    x = np.ascontiguousarray(np.asarray(x, dtype=np.float32))
    prev_hidden = np.asarray(prev_hidden, dtype=np.float32)
    f32c = lambda a: np.ascontiguousarray(np.asarray(a, dtype=np.float32))
    W1, b1, W2, b2, Wg, Ug, bg, Wp, bp, Wv, bv = map(
        f32c, (W1, b1, W2, b2, Wg, Ug, bg, Wp, bp, Wv, bv))

    if "nc" not in _cache:
        _cache["nc"] = _build_program()
    nc = _cache["nc"]

    xTpad = np.zeros((D_IN, WARM + T), np.float16)
    xTpad[:, WARM:] = x.T.astype(np.float16)
    bXp = (bg[0] + np.concatenate([bg[1][:2 * H], np.zeros(H, np.float32)])
           ).reshape(3 * H, 1)
    brh = bg[1][2 * H:].reshape(H, 1)
    Wpv = np.concatenate([Wp, Wv], axis=1)
    bpv = np.tile(np.concatenate([bp, bv])[None, :], (128, 1)).astype(np.float32)

    common = {
        "W1d": W1.astype(np.float16), "W2d": W2.astype(np.float16),
        "Wgd": Wg.astype(np.float16), "Ugd": Ug,
        "Ug16d": Ug.astype(np.float16),
        "b1d": b1.reshape(D, 1), "b2d": b2.reshape(D, 1),
        "bXpd": bXp, "brhd": brh, "Wpvd": Wpv, "bpvd": bpv,
        "idd": np.eye(128, dtype=np.float16),
    }
    in_maps = []
    for c in range(N_CORES):
        mf = np.empty((H, 2), np.float32)
        if c == 0:
            mf[:, 0] = 0.0
            mf[:, 1] = prev_hidden[0]
        else:
            mf[:, 0] = 1.0
            mf[:, 1] = 0.0
        in_maps.append({
            **common,
            "xT": np.ascontiguousarray(xTpad[:, c * TC:c * TC + NL]),
            "mfd": mf,
        })

    trace = os.environ.get("BASS_KERNEL_TRACE") == "1"
    try:
        res = run_bass_kernel_spmd(nc, in_maps, list(range(N_CORES)), trace=trace)
        if trace:
            _last_exec_ns = res.exec_time_ns
            _last_scope_times = res.per_core_scope_times
        results = res.results
    except Exception:
        # Transient NRT faults (e.g. EXEC_UNIT_UNRECOVERABLE on first exec of a
        # fresh NEFF) poison the in-process PJRT client; retry in clean
        # subprocesses, which also warm the NEFF cache for a final attempt.
        results = _run_in_subprocess(in_maps)

    policy = np.concatenate([results[c]["policy"] for c in range(N_CORES)], axis=0)
    value = np.concatenate([results[c]["value"] for c in range(N_CORES)], axis=0)
    hT = results[N_CORES - 1]["hT"].reshape(1, H)
    return policy, value, hT


def _run_in_subprocess(in_maps):
    import pickle
    import subprocess
    import tempfile
    import time as _time

    last = None
    for attempt in range(3):
        with tempfile.TemporaryDirectory() as td:
            inp = os.path.join(td, "in.pkl")
            outp = os.path.join(td, "out.pkl")
            with open(inp, "wb") as f:
                pickle.dump(in_maps, f)
            code = (
                "import pickle, importlib.util, sys\n"
                f"spec = importlib.util.spec_from_file_location('knl', {__file__!r})\n"
                "m = importlib.util.module_from_spec(spec); spec.loader.exec_module(m)\n"
                f"in_maps = pickle.load(open({'PLACEIN'!r}, 'rb'))\n"
                "from concourse.bass_utils import run_bass_kernel_spmd\n"
                "nc = m._build_program()\n"
                "res = run_bass_kernel_spmd(nc, in_maps, list(range(m.N_CORES)))\n"
                f"pickle.dump(res.results, open({'PLACEOUT'!r}, 'wb'))\n"
            ).replace("PLACEIN", inp).replace("PLACEOUT", outp)
            r = subprocess.run([sys.executable, "-c", code], capture_output=True)
            if r.returncode == 0 and os.path.exists(outp):
                with open(outp, "rb") as f:
                    return pickle.load(f)
            last = r.stderr.decode(errors="replace")[-2000:]
            _time.sleep(5)
    raise RuntimeError(f"device run failed after retries: {last}")


# revision 17
# speedup vs baseline: 1.2914x; 1.0343x over previous
"""Trainium2 Bass kernel for nn_ActorCritic_MLPLSTM (MLP front-end + GRU +
actor/critic heads), distributed over 8 NeuronCores.

Algorithm
---------
The GRU recurrence h_t = z_t*h_{t-1} + (1-z_t)*hh_t (reset_after, sigmoid
candidate) is solved by fixed-point iteration ("scan-Picard"): each sweep
recomputes the recurrent projections rec = h_prev @ Ug for ALL timesteps as one
batched matmul, forms the gates, then solves the *diagonal linear* recurrence
h_t = a_t*h_{t-1} + b_t exactly with the hardware tensor_tensor_scan
instruction. The nonlinear fixed point converges at ~9x error reduction per
sweep. Sweeps run with fp16 matmuls (4x faster than fp32 on the PE, 10x more
accurate than bf16); the gate input projections are folded into the PSUM
accumulation with an fp16 identity matmul so the sigmoids read PSUM directly.
Final rel err ~1.5e-4 (fp16-rounding floor of the MLP front-end).

Sharding: time dimension split across 8 cores (1024 steps each) with a
192-step warmup prefix per core. The GRU's z-gate product over >=128 steps
attenuates any entry-state error to ~0, so the cores need NO communication:
each core's warmup region converges to the true trajectory before its owned
steps begin. Core 0 instead forces its known initial state via a masked fixup
of the scan coefficients at the warmup boundary.

Everything on-chip lives in "feature-major" layout [feature -> partitions,
time -> free dim], so matmuls contract over partitions and the scan runs along
the free dim. Inputs x are transposed on the host as part of sharding.
"""

import os
import sys

import numpy as np

for _p in ("/opt/trn_rl_repo", "/root/.axon_site/_ro/trn_rl_repo"):
    if os.path.isdir(_p) and _p not in sys.path:
        sys.path.append(_p)

T, D_IN, D, H, A = 8192, 2048, 512, 512, 18
N_CORES = 8
TC = T // N_CORES            # 1024 owned steps per core
WARM = 64                    # warmup prefix steps
NL = WARM + TC               # 1216 local steps per core
N_SLICES = ((0, 512), (512, 512), (1024, NL - 1024))  # (start, size) free-dim slices
N_F16_SWEEPS = 5
N_FP32_SWEEPS = 0

_cache = {}
_last_exec_ns = None
_last_scope_times = None


def _build_program():
    import concourse.bacc as bacc
    import concourse.mybir as mybir
    from concourse import tile

    f32 = mybir.dt.float32
    f16 = mybir.dt.float16
    ACT = mybir.ActivationFunctionType
    ALU = mybir.AluOpType
    AXX = mybir.AxisListType.X

    nc = bacc.Bacc("TRN2", target_bir_lowering=False, debug=False,
                   num_devices=N_CORES)

    xT_d = nc.dram_tensor("xT", [D_IN, NL], f16, kind="ExternalInput")
    W1_d = nc.dram_tensor("W1d", [D_IN, D], f16, kind="ExternalInput")
    W2_d = nc.dram_tensor("W2d", [D, D], f16, kind="ExternalInput")
    Wg_d = nc.dram_tensor("Wgd", [D, 3 * H], f16, kind="ExternalInput")
    Ug_d = nc.dram_tensor("Ugd", [H, 3 * H], f32, kind="ExternalInput")
    Ug16_d = nc.dram_tensor("Ug16d", [H, 3 * H], f16, kind="ExternalInput")
    b1_d = nc.dram_tensor("b1d", [D, 1], f32, kind="ExternalInput")
    b2_d = nc.dram_tensor("b2d", [D, 1], f32, kind="ExternalInput")
    bXp_d = nc.dram_tensor("bXpd", [3 * H, 1], f32, kind="ExternalInput")
    brh_d = nc.dram_tensor("brhd", [H, 1], f32, kind="ExternalInput")
    mf_d = nc.dram_tensor("mfd", [H, 2], f32, kind="ExternalInput")
    Wpv_d = nc.dram_tensor("Wpvd", [H, A + 1], f32, kind="ExternalInput")
    bpv_d = nc.dram_tensor("bpvd", [128, A + 1], f32, kind="ExternalInput")
    id_d = nc.dram_tensor("idd", [128, 128], f16, kind="ExternalInput")

    pol_d = nc.dram_tensor("policy", [TC, A], f32, kind="ExternalOutput")
    val_d = nc.dram_tensor("value", [TC, 1], f32, kind="ExternalOutput")
    hT_d = nc.dram_tensor("hT", [H, 1], f32, kind="ExternalOutput")

    KD = D // 128        # 4  k-tiles over D/H
    KIN = D_IN // 128    # 16 k-tiles over D_IN
    M3H = 3 * H // 128   # 12 m-tiles over 3H

    with tile.TileContext(nc) as tc:
        with tc.tile_pool(name="resident", bufs=1) as rp:
            XT = [rp.tile([128, NL], f16 if m < 2 * KD else f32,
                          name=f"XT{m}", tag=f"XT{m}") for m in range(M3H)]
            id16 = rp.tile([128, 128], f16, name="id16", tag="id16")
            nc.gpsimd.dma_start(id16[:], id_d[:])
            hb16 = [[rp.tile([128, NL + 1], f16, name=f"hq{b}_{k}", tag=f"hq{b}_{k}")
                     for k in range(KD)] for b in range(2)]
            brh_sb = rp.tile([128, KD], f32, name="brh", tag="brh")
            mf_sb = rp.tile([128, 2 * KD], f32, name="mf", tag="mf")
            for k in range(KD):
                nc.gpsimd.dma_start(brh_sb[:, k:k + 1], brh_d[k * 128:(k + 1) * 128, :])
                nc.gpsimd.dma_start(mf_sb[:, 2 * k:2 * k + 2],
                                    mf_d[k * 128:(k + 1) * 128, :])
                nc.vector.memset(hb16[0][k][:, 0:1], 0.0)
                nc.vector.memset(hb16[1][k][:, 0:1], 0.0)
            Wpv_sb = [rp.tile([128, A + 1], f32, name=f"wpv{k}", tag=f"wpv{k}")
                      for k in range(KD)]
            bpv_sb = rp.tile([128, A + 1], f32, name="bpv", tag="bpv")
            for k in range(KD):
                nc.gpsimd.dma_start(Wpv_sb[k][:], Wpv_d[k * 128:(k + 1) * 128, :])
            nc.gpsimd.dma_start(bpv_sb[:], bpv_d[:])

            # ---------------- Phase A: MLP front-end + gate input projections
            with nc.named_scope("phaseA"), \
                 tc.tile_pool(name="wA", bufs=1) as wA, \
                 tc.tile_pool(name="sA", bufs=6) as sA, \
                 tc.tile_pool(name="zA", bufs=2) as zA, \
                 tc.tile_pool(name="psA", bufs=1, space="PSUM") as psA:
                W1_sb = [wA.tile([128, D], f16, name=f"w1_{k}", tag=f"w1_{k}")
                         for k in range(KIN)]
                W2_sb = [wA.tile([128, D], f16, name=f"w2_{k}", tag=f"w2_{k}")
                         for k in range(KD)]
                Wg_sb = [wA.tile([128, 3 * H], f16, name=f"wg_{k}", tag=f"wg_{k}")
                         for k in range(KD)]
                b1_sb = wA.tile([128, KD], f32, name="b1", tag="b1")
                b2_sb = wA.tile([128, KD], f32, name="b2", tag="b2")
                bXp_sb = wA.tile([128, M3H], f32, name="bXp", tag="bXp")
                for k in range(KIN):
                    nc.sync.dma_start(W1_sb[k][:], W1_d[k * 128:(k + 1) * 128, :])
                for k in range(KD):
                    nc.sync.dma_start(b1_sb[:, k:k + 1], b1_d[k * 128:(k + 1) * 128, :])
                    nc.gpsimd.dma_start(b2_sb[:, k:k + 1], b2_d[k * 128:(k + 1) * 128, :])
                for k in range(KD):
                    nc.gpsimd.dma_start(W2_sb[k][:], W2_d[k * 128:(k + 1) * 128, :])
                    nc.gpsimd.dma_start(Wg_sb[k][:], Wg_d[k * 128:(k + 1) * 128, :])
                for m in range(M3H):
                    nc.gpsimd.dma_start(bXp_sb[:, m:m + 1], bXp_d[m * 128:(m + 1) * 128, :])

                for (st, ns) in N_SLICES:
                    # z1 = relu(W1^T xT + b1)  [D-major, ns]
                    z1s = [zA.tile([128, 512], f16, name=f"z1_{m}", tag=f"z1_{m}")
                           for m in range(KD)]
                    ps1 = [psA.tile([128, ns], f32, name=f"psA{m}", tag=f"psA{m}")
                           for m in range(KD)]
                    for k in range(KIN):
                        xt = sA.tile([128, 512], f16, name="xt", tag="xt")
                        nc.sync.dma_start(xt[:, :ns],
                                          xT_d[k * 128:(k + 1) * 128, st:st + ns])
                        for m in range(KD):
                            nc.tensor.matmul(ps1[m][:],
                                             W1_sb[k][:, m * 128:(m + 1) * 128],
                                             xt[:, :ns],
                                             start=(k == 0), stop=(k == KIN - 1))
                    for m in range(KD):
                        nc.scalar.activation(z1s[m][:, :ns], ps1[m][:], ACT.Relu,
                                             bias=b1_sb[:, m:m + 1])
                    # z2 = relu(W2^T z1 + b2)
                    z2s = [zA.tile([128, 512], f16, name=f"z2_{m}", tag=f"z2_{m}")
                           for m in range(KD)]
                    ps2 = [psA.tile([128, ns], f32, name=f"psA{m}", tag=f"psA{m}")
                           for m in range(KD)]
                    for k in range(KD):
                        for m in range(KD):
                            nc.tensor.matmul(ps2[m][:],
                                             W2_sb[k][:, m * 128:(m + 1) * 128],
                                             z1s[k][:, :ns],
                                             start=(k == 0), stop=(k == KD - 1))
                    for m in range(KD):
                        nc.scalar.activation(z2s[m][:, :ns], ps2[m][:], ACT.Relu,
                                             bias=b2_sb[:, m:m + 1])
                    # XT = Wg^T z2 + (bg0 + [bg1_zr; 0])   (two psum half-waves)
                    for half in range(2):
                        ms = range(6 * half, 6 * half + 6)
                        psX = {m: psA.tile([128, ns], f32, name=f"psA{m - 6 * half}",
                                           tag=f"psA{m - 6 * half}") for m in ms}
                        for k in range(KD):
                            for m in ms:
                                nc.tensor.matmul(psX[m][:],
                                                 Wg_sb[k][:, m * 128:(m + 1) * 128],
                                                 z2s[k][:, :ns],
                                                 start=(k == 0), stop=(k == KD - 1))
                        for m in ms:
                            nc.scalar.activation(XT[m][:, st:st + ns], psX[m][:],
                                                 ACT.Identity, bias=bXp_sb[:, m:m + 1])

                    # -------- fused sweep 0: trajectory == 0, so rec == 0 and
                    # the gates read X directly; scans write hb16[1]
                    s0a, s0g, s0r, s0h, s0b = {}, {}, {}, {}, {}
                    for k in range(KD):
                        a_sl = zA.tile([128, 512], f32, name=f"s0a{k}", tag=f"s0a{k}")
                        g_sl = zA.tile([128, 512], f32, name=f"s0g{k}", tag=f"s0g{k}")
                        nc.scalar.activation(a_sl[:, :ns], XT[k][:, st:st + ns],
                                             ACT.Sigmoid)
                        nc.vector.tensor_scalar(g_sl[:, :ns], a_sl[:, :ns], -1.0, 1.0,
                                                op0=ALU.mult, op1=ALU.add)
                        s0a[k] = a_sl
                        s0g[k] = g_sl
                        rt = zA.tile([128, 512], f32, name=f"s0r{k}", tag=f"s0r{k}")
                        nc.scalar.activation(rt[:, :ns], XT[KD + k][:, st:st + ns],
                                             ACT.Sigmoid)
                        s0r[k] = rt
                    for k in range(KD):
                        pre = zA.tile([128, 512], f32, name=f"s0p{k}", tag=f"s0p{k}")
                        hh = zA.tile([128, 512], f32, name=f"s0h{k}", tag=f"s0h{k}")
                        nc.vector.tensor_scalar(pre[:, :ns], s0r[k][:, :ns],
                                                brh_sb[:, k:k + 1], None, op0=ALU.mult)
                        nc.vector.tensor_add(pre[:, :ns], pre[:, :ns],
                                             XT[2 * KD + k][:, st:st + ns])
                        nc.scalar.activation(hh[:, :ns], pre[:, :ns], ACT.Sigmoid)
                        b_sl = zA.tile([128, 512], f32, name=f"s0b{k}", tag=f"s0b{k}")
                        nc.vector.tensor_mul(b_sl[:, :ns], s0g[k][:, :ns], hh[:, :ns])
                        s0b[k] = b_sl
                    if st == 0:
                        c = WARM - 1
                        for k in range(KD):
                            nc.vector.tensor_scalar(
                                s0a[k][:, c:c + 1], s0a[k][:, c:c + 1],
                                mf_sb[:, 2 * k:2 * k + 1], None, op0=ALU.mult)
                            nc.vector.tensor_scalar(
                                s0b[k][:, c:c + 1], s0b[k][:, c:c + 1],
                                mf_sb[:, 2 * k:2 * k + 1],
                                mf_sb[:, 2 * k + 1:2 * k + 2],
                                op0=ALU.mult, op1=ALU.add)
                    for k in range(KD):
                        nc.vector.tensor_tensor_scan(
                            hb16[1][k][:, 1 + st:1 + st + ns],
                            s0a[k][:, :ns], s0b[k][:, :ns],
                            hb16[1][k][:, st:st + 1], ALU.mult, ALU.add)

            # ---------------- Phase B: scan-Picard sweeps (bf16 then fp32)
            with tc.tile_pool(name="rpB", bufs=1) as rpB, \
                 tc.tile_pool(name="sB", bufs=4) as sB, \
                 tc.tile_pool(name="sab", bufs=1) as sab, \
                 tc.tile_pool(name="srt", bufs=1) as srt, \
                 tc.tile_pool(name="psB", bufs=7, space="PSUM") as psB:
                n_hb = max(1, 2 * min(N_FP32_SWEEPS, 1))
                hb = [[rpB.tile([128, NL + 1], f32, name=f"hb{b}_{k}", tag=f"hb{b}_{k}")
                       for k in range(KD)] for b in range(n_hb)]
                Ug_sb = [rpB.tile([128, 3 * H], f32, name=f"ug{k}", tag=f"ug{k}")
                         for k in range(KD)] if N_FP32_SWEEPS else None
                Ug16 = [rpB.tile([128, 3 * H], f16, name=f"uq{k}", tag=f"uq{k}")
                        for k in range(KD)]
                for k in range(KD):
                    nc.sync.dma_start(Ug16[k][:], Ug16_d[k * 128:(k + 1) * 128, :])
                    if N_FP32_SWEEPS:
                        nc.sync.dma_start(Ug_sb[k][:], Ug_d[k * 128:(k + 1) * 128, :])
                    for b in range(n_hb):
                        nc.vector.memset(hb[b][k][:, 0:1], 0.0)

                # (input_buffer, output_buffer, matmul_weights) per sweep
                schedule = []
                for s in range(1, N_F16_SWEEPS):
                    inb = hb16[s % 2]
                    outb = hb16[(s + 1) % 2] if s < N_F16_SWEEPS - 1 else hb[0]
                    schedule.append((s, inb, outb, Ug16))
                for s in range(N_FP32_SWEEPS):
                    schedule.append((N_F16_SWEEPS + s, hb[s % 2], hb[(s + 1) % 2], Ug_sb))
                hfin = hb[N_FP32_SWEEPS % 2]

                def emit_head(tt, sC, psum_pool):
                    col = 1 + WARM + tt * 128
                    ps = psum_pool.tile([128, A + 1], f32, name="psC", tag="psC", bufs=1)
                    for k in range(KD):
                        nc.tensor.matmul(ps[:], hfin[k][:, col:col + 128],
                                         Wpv_sb[k][:], start=(k == 0),
                                         stop=(k == KD - 1))
                    L = sC.tile([128, A + 1], f32, name="L", tag="L")
                    nc.vector.tensor_add(L[:], ps[:], bpv_sb[:])
                    nmx = sC.tile([128, 1], f32, name="nmx", tag="nmx")
                    nc.vector.reduce_max(nmx[:], L[:, :A], axis=AXX, negate=True)
                    E = sC.tile([128, A], f32, name="E", tag="E")
                    nc.scalar.activation(E[:], L[:, :A], ACT.Exp, bias=nmx[:])
                    sm = sC.tile([128, 1], f32, name="sm", tag="sm")
                    nc.vector.reduce_sum(sm[:], E[:], axis=AXX)
                    rs = sC.tile([128, 1], f32, name="rs", tag="rs")
                    nc.vector.reciprocal(rs[:], sm[:])
                    P = sC.tile([128, A], f32, name="P", tag="P")
                    nc.vector.tensor_scalar(P[:], E[:], rs[:], None, op0=ALU.mult)
                    nc.sync.dma_start(pol_d[tt * 128:(tt + 1) * 128, :], P[:])
                    nc.sync.dma_start(val_d[tt * 128:(tt + 1) * 128, :],
                                      L[:, A:A + 1])

                # head t-tiles grouped by which scan slice covers their columns,
                # emitted one slice late so the PE never waits on a fresh scan
                head_groups = {1: [0, 1, 2], 2: [3, 4, 5, 6], 3: [7]}
                last_s = schedule[-1][0]
                with tc.tile_pool(name="sC", bufs=4) as sC:
                  for s, cur, nxt, Ugs in schedule:
                    with nc.named_scope(f"sweep{s}"):
                        for (st, ns) in N_SLICES:
                            ats = {}
                            gts = {}
                            rts = {}
                            hhs = {}
                            bts = {}
                            for m in range(M3H):
                                ps = psB.tile([128, ns], f32, name="psB", tag="psB")
                                fold = m < 2 * KD
                                for k in range(KD):
                                    nc.tensor.matmul(ps[:],
                                                     Ugs[k][:, m * 128:(m + 1) * 128],
                                                     cur[k][:, st:st + ns],
                                                     start=(k == 0),
                                                     stop=(k == KD - 1 and not fold))
                                if fold:
                                    nc.tensor.matmul(ps[:], id16[:],
                                                     XT[m][:, st:st + ns],
                                                     start=False, stop=True)
                                if m < KD:        # z gate -> a coefficients
                                    a_sl = sab.tile([128, 512], f32, name=f"a{m}",
                                                    tag=f"a{m}")
                                    g_sl = sab.tile([128, 512], f32, name=f"g{m}",
                                                    tag=f"g{m}")
                                    nc.scalar.activation(a_sl[:, :ns], ps[:],
                                                         ACT.Sigmoid)
                                    nc.vector.tensor_scalar(g_sl[:, :ns], a_sl[:, :ns],
                                                            -1.0, 1.0,
                                                            op0=ALU.mult, op1=ALU.add)
                                    ats[m] = a_sl
                                    gts[m] = g_sl
                                elif m < 2 * KD:  # r gate
                                    k0 = m - KD
                                    rt = srt.tile([128, 512], f32, name=f"rt{k0}",
                                                  tag=f"rt{k0}")
                                    nc.scalar.activation(rt[:, :ns], ps[:],
                                                         ACT.Sigmoid)
                                    rts[k0] = rt
                                else:             # hh = sig(Xh + rt*(rec+brh))
                                    k0 = m - 2 * KD
                                    pre = sB.tile([128, 512], f32, name="pre", tag="pre")
                                    hh = srt.tile([128, 512], f32, name=f"hh{k0}",
                                                  tag=f"hh{k0}")
                                    nc.vector.scalar_tensor_tensor(
                                        pre[:, :ns], ps[:], brh_sb[:, k0:k0 + 1],
                                        rts[k0][:, :ns], op0=ALU.add, op1=ALU.mult)
                                    nc.vector.tensor_add(pre[:, :ns], pre[:, :ns],
                                                         XT[m][:, st:st + ns])
                                    nc.scalar.activation(hh[:, :ns], pre[:, :ns],
                                                         ACT.Sigmoid)
                                    hhs[k0] = hh
                            for k in range(KD):   # b = (1-z)*hh, 1-z = sig(-pre)
                                b_sl = sab.tile([128, 512], f32, name=f"b{k}",
                                                tag=f"b{k}")
                                nc.vector.tensor_mul(b_sl[:, :ns], gts[k][:, :ns],
                                                     hhs[k][:, :ns])
                                bts[k] = b_sl
                            if st == 0:
                                # force scan state entering the first owned step:
                                # core 0: a=0, b=h0; other cores: no-op
                                c = WARM - 1
                                for k in range(KD):
                                    nc.vector.tensor_scalar(
                                        ats[k][:, c:c + 1], ats[k][:, c:c + 1],
                                        mf_sb[:, 2 * k:2 * k + 1], None, op0=ALU.mult)
                                    nc.vector.tensor_scalar(
                                        bts[k][:, c:c + 1], bts[k][:, c:c + 1],
                                        mf_sb[:, 2 * k:2 * k + 1],
                                        mf_sb[:, 2 * k + 1:2 * k + 2],
                                        op0=ALU.mult, op1=ALU.add)
                            for k in range(KD):
                                nc.vector.tensor_tensor_scan(
                                    nxt[k][:, 1 + st:1 + st + ns],
                                    ats[k][:, :ns], bts[k][:, :ns],
                                    nxt[k][:, st:st + 1], ALU.mult, ALU.add)
                            if s == last_s:
                                sl_i = [x0 for x0, (s0_, _) in enumerate(N_SLICES)
                                        if s0_ == st][0] + 1
                                if sl_i >= 2:
                                    for tt in head_groups[sl_i - 1]:
                                        emit_head(tt, sC, psB)
                                if sl_i == 3:
                                    for tt in head_groups[3]:
                                        emit_head(tt, sC, psB)
                                    for k in range(KD):
                                        nc.sync.dma_start(
                                            hT_d[k * 128:(k + 1) * 128, :],
                                            hfin[k][:, NL:NL + 1])

    nc.compile()
    return nc


def kernel(x, prev_hidden, W1, b1, W2, b2, Wg, Ug, bg, Wp, bp, Wv, bv):
    global _last_exec_ns, _last_scope_times
    from concourse.bass_utils import run_bass_kernel_spmd

    x = np.ascontiguousarray(np.asarray(x, dtype=np.float32))
    prev_hidden = np.asarray(prev_hidden, dtype=np.float32)
    f32c = lambda a: np.ascontiguousarray(np.asarray(a, dtype=np.float32))
    W1, b1, W2, b2, Wg, Ug, bg, Wp, bp, Wv, bv = map(
        f32c, (W1, b1, W2, b2, Wg, Ug, bg, Wp, bp, Wv, bv))

    if "nc" not in _cache:
        _cache["nc"] = _build_program()
    nc = _cache["nc"]

    xTpad = np.zeros((D_IN, WARM + T), np.float16)
    xTpad[:, WARM:] = x.T.astype(np.float16)
    bXp = (bg[0] + np.concatenate([bg[1][:2 * H], np.zeros(H, np.float32)])
           ).reshape(3 * H, 1)
    brh = bg[1][2 * H:].reshape(H, 1)
    Wpv = np.concatenate([Wp, Wv], axis=1)
    bpv = np.tile(np.concatenate([bp, bv])[None, :], (128, 1)).astype(np.float32)

    common = {
        "W1d": W1.astype(np.float16), "W2d": W2.astype(np.float16),
        "Wgd": Wg.astype(np.float16), "Ugd": Ug,
        "Ug16d": Ug.astype(np.float16),
        "b1d": b1.reshape(D, 1), "b2d": b2.reshape(D, 1),
        "bXpd": bXp, "brhd": brh, "Wpvd": Wpv, "bpvd": bpv,
        "idd": np.eye(128, dtype=np.float16),
    }
    in_maps = []
    for c in range(N_CORES):
        mf = np.empty((H, 2), np.float32)
        if c == 0:
            mf[:, 0] = 0.0
            mf[:, 1] = prev_hidden[0]
        else:
            mf[:, 0] = 1.0
            mf[:, 1] = 0.0
        in_maps.append({
            **common,
            "xT": np.ascontiguousarray(xTpad[:, c * TC:c * TC + NL]),
            "mfd": mf,
        })

    trace = os.environ.get("BASS_KERNEL_TRACE") == "1"
    try:
        res = run_bass_kernel_spmd(nc, in_maps, list(range(N_CORES)), trace=trace)
        if trace:
            _last_exec_ns = res.exec_time_ns
            _last_scope_times = res.per_core_scope_times
        results = res.results
    except Exception:
        # Transient NRT faults (e.g. EXEC_UNIT_UNRECOVERABLE on first exec of a
        # fresh NEFF) poison the in-process PJRT client; retry in clean
        # subprocesses, which also warm the NEFF cache for a final attempt.
        results = _run_in_subprocess(in_maps)

    policy = np.concatenate([results[c]["policy"] for c in range(N_CORES)], axis=0)
    value = np.concatenate([results[c]["value"] for c in range(N_CORES)], axis=0)
    hT = results[N_CORES - 1]["hT"].reshape(1, H)
    return policy, value, hT


def _run_in_subprocess(in_maps):
    import pickle
    import subprocess
    import tempfile
    import time as _time

    last = None
    for attempt in range(3):
        with tempfile.TemporaryDirectory() as td:
            inp = os.path.join(td, "in.pkl")
            outp = os.path.join(td, "out.pkl")
            with open(inp, "wb") as f:
                pickle.dump(in_maps, f)
            code = (
                "import pickle, importlib.util, sys\n"
                f"spec = importlib.util.spec_from_file_location('knl', {__file__!r})\n"
                "m = importlib.util.module_from_spec(spec); spec.loader.exec_module(m)\n"
                f"in_maps = pickle.load(open({'PLACEIN'!r}, 'rb'))\n"
                "from concourse.bass_utils import run_bass_kernel_spmd\n"
                "nc = m._build_program()\n"
                "res = run_bass_kernel_spmd(nc, in_maps, list(range(m.N_CORES)))\n"
                f"pickle.dump(res.results, open({'PLACEOUT'!r}, 'wb'))\n"
            ).replace("PLACEIN", inp).replace("PLACEOUT", outp)
            r = subprocess.run([sys.executable, "-c", code], capture_output=True)
            if r.returncode == 0 and os.path.exists(outp):
                with open(outp, "rb") as f:
                    return pickle.load(f)
            last = r.stderr.decode(errors="replace")[-2000:]
            _time.sleep(5)
    raise RuntimeError(f"device run failed after retries: {last}")
